# revision 3
# baseline (speedup 1.0000x reference)
"""Trainium2 Bass kernel for nn_KATLayer (KAT basis-function layer).

out[b,o] = sum_{i,n} exp(-z^2) * (1 + erf(alpha*z/sqrt(2))) * w[i,o,n]
  z = (x[b,i] - c[i,o,n]) / (|sigma|+1e-8),  c = |scale|*mx_start + mx_train

Sharding: output dim O split across 8 cores (O_shard=64). Per core:
  partitions = i (4 chunks of 128), free = (o_local, n) = 1024 per tile,
  tiles processed in PAIRS (2 consecutive b, same i-chunk) so the
  elementwise/activation ops run at free=2048 to amortize fixed overheads.

Math restructure vs the naive pipeline (all products fp16, z kept fp16 —
validated 5.2e-4 rel err vs 2e-2 gate):
  DVE:  zm = (c - x)*rinv          [STT fp32-in -> fp16 out; = -z]
  ACT:  e  = Derivative_Erf(zm)    [= 2/sqrt(pi)*exp(-z^2), even in z]
  DVE:  q  = e*wt                  [wt = w*sqrt(pi)/2 -> q = w*exp(-z^2)]
  DVE:  um = zm*A3                 [A3 = alpha/sqrt(2); fp16 TT 2x mode]
  ACT:  t  = Erf(-um)              [= erf(alpha*z/sqrt(2))]
  DVE:  r  = q*t
  PE :  psum += onehot_b.T @ q ; psum += onehot_b.T @ r
        (the "+1" of (1+erf) is absorbed by accumulating BOTH the q and r
        streams in PSUM, killing the fp16 STT (1x-only uop) of the old
        s=(t+1)*wt formulation)
Final: DVE reduce over n: psum(32,64,16) -> (32,64); DMA out.

um/q/r run as fp16 tensor_tensor (2x mode, 2 elem/cyc/lane); per-k consts
(A3, wt) are read through stride-0 broadcast APs across the pair dim.

Derivative_Erf and Erf live in different ACT table sets (~2.7us/switch), so
pairs are processed in groups with phase-batched activations (2 switches per
group).
"""
import sys

sys.path.insert(0, "/opt/trn_rl_repo")
import math

import numpy as np

B, I, O, N = 32, 512, 512, 16
NCORES = 8
OS = O // NCORES          # 64 output dims per core
KC = I // 128             # 4 i-chunks
P = 128
GQ = 6                    # pairs per activation-phase group (12 tiles)
INV_SQRT2 = 0.7071067811865476
SQRT_PI_2 = math.sqrt(math.pi) / 2.0

_CACHE = {}
LAST_RESULTS = None


def _build_nc(reps=1, GQ=GQ):
    import concourse.bacc as bacc
    import concourse.mybir as mybir
    from concourse import tile

    fp32 = mybir.dt.float32
    fp16 = mybir.dt.float16
    AF = mybir.ActivationFunctionType
    ALU = mybir.AluOpType

    nc = bacc.Bacc(
        "TRN2", target_bir_lowering=False, debug=False, num_devices=NCORES
    )
    c_d = nc.dram_tensor("c", [KC, P, OS, N], fp32, kind="ExternalInput")
    r_d = nc.dram_tensor("r", [KC, P, OS, N], fp32, kind="ExternalInput")
    a_d = nc.dram_tensor("a", [KC, P, OS, N], fp16, kind="ExternalInput")
    w_d = nc.dram_tensor("w", [KC, P, OS, N], fp16, kind="ExternalInput")
    x_d = nc.dram_tensor("x", [KC, P, B], fp32, kind="ExternalInput")
    oh_d = nc.dram_tensor("oh", [P, B, B], fp16, kind="ExternalInput")
    out_d = nc.dram_tensor("out", [B, OS], fp32, kind="ExternalOutput")

    with tile.TileContext(nc) as tc:
        with (
            tc.tile_pool(name="const", bufs=1) as cpool,
            tc.tile_pool(name="zp", bufs=GQ + 2) as zpool,
            tc.tile_pool(name="qp", bufs=GQ + 2) as qpool,
            tc.tile_pool(name="tp", bufs=GQ + 2) as tpool,
            tc.tile_pool(name="ep", bufs=3) as epool,
            tc.tile_pool(name="up", bufs=3) as upool,
            tc.tile_pool(name="rp", bufs=3) as rpool,
            tc.tile_pool(name="psum", bufs=1, space="PSUM") as psp,
            tc.tile_pool(name="outp", bufs=1) as opool,
        ):
            c_sb, r_sb, a_sb, w_sb = [], [], [], []
            for k in range(KC):
                for lst, dram, nm, dt_ in (
                    (c_sb, c_d, "c", fp32),
                    (r_sb, r_d, "r", fp32),
                    (a_sb, a_d, "a", fp16),
                    (w_sb, w_d, "w", fp16),
                ):
                    t = cpool.tile([P, OS, N], dt_, tag=f"{nm}{k}")
                    nc.sync.dma_start(t[:], dram[k])
                    lst.append(t)
            x_sb = cpool.tile([P, KC * B], fp32, tag="x")
            for k in range(KC):
                nc.sync.dma_start(x_sb[:, k * B : (k + 1) * B], x_d[k])
            oh_sb = cpool.tile([P, B, B], fp16, tag="oh")
            nc.sync.dma_start(oh_sb[:], oh_d[:])

            psum_t = psp.tile([B, OS, N], fp32)
            # pairs: 2 consecutive b, same k
            pairs = [(k, b) for k in range(KC) for b in range(0, B, 2)]
            n_pairs = len(pairs)
            out_sb = opool.tile([B, OS], fp32)

            def bcast2(t):
                return t[:, None].broadcast_to((P, 2, OS, N))

            for rep in range(reps):
                n_mm = 0
                total_mm = 8 * n_pairs
                for g0 in range(0, n_pairs, GQ):
                    grp = pairs[g0 : g0 + GQ]
                    # phase 1: zm for all pairs in group (DVE STT, fp32->fp16)
                    zms = []
                    for k, b in grp:
                        zm = zpool.tile([P, 2, OS, N], fp16, tag="zp")
                        for j in range(2):
                            xcol = x_sb[:, k * B + b + j : k * B + b + j + 1]
                            nc.vector.scalar_tensor_tensor(
                                zm[:, j], c_sb[k][:], xcol, r_sb[k][:],
                                op0=ALU.subtract, op1=ALU.mult,
                            )
                        zms.append(zm)
                    # phase 2: e = D_ERF(zm) (ACT, table A); q = e*wt (DVE)
                    qs = []
                    for (k, b), zm in zip(grp, zms):
                        e = epool.tile([P, 2, OS, N], fp16)
                        nc.scalar.activation(e[:], zm[:], AF.Derivative_Erf)
                        q = qpool.tile([P, 2, OS, N], fp16, tag="qp")
                        nc.vector.tensor_mul(q[:], e[:], bcast2(w_sb[k]))
                        qs.append(q)
                    # phase 3: um = zm*A3 (GpSimd); t = Erf(-um) (ACT, table B)
                    ts_ = []
                    for (k, b), zm in zip(grp, zms):
                        um = upool.tile([P, 2, OS, N], fp16)
                        nc.gpsimd.tensor_mul(um[:], zm[:], bcast2(a_sb[k]))
                        t_ = tpool.tile([P, 2, OS, N], fp16, tag="tp")
                        nc.scalar.activation(t_[:], um[:], AF.Erf, scale=-1.0)
                        ts_.append(t_)
                    # phase 4: r = q*t (DVE); accumulate q and r streams (PE)
                    for (k, b), q, t_ in zip(grp, qs, ts_):
                        r_ = rpool.tile([P, 2, OS, N], fp16)
                        nc.vector.tensor_mul(r_[:], q[:], t_[:])
                        for j in range(2):
                            for src in (q, r_):
                                for h in range(2):
                                    nc.tensor.matmul(
                                        psum_t[:, 32 * h : 32 * (h + 1), :],
                                        oh_sb[:, b + j, :],
                                        src[:, j, 32 * h : 32 * (h + 1), :],
                                        start=(n_mm < 2),
                                        stop=(n_mm >= total_mm - 2),
                                    )
                                    n_mm += 1

                nc.vector.tensor_reduce(
                    out_sb[:], psum_t[:], axis=mybir.AxisListType.X, op=ALU.add
                )
            nc.sync.dma_start(out_d[:], out_sb[:])

    nc.compile()
    return nc


def _prep_inputs(x, mx_train, scale, sigma, alpha, w, mx_start):
    c = (np.abs(scale)[:, :, None] * mx_start[None, None, :]
         + mx_train[:, :, None]).astype(np.float32)
    rinv = (1.0 / (np.abs(sigma) + 1e-8)).astype(np.float32)
    A3 = (alpha * INV_SQRT2).astype(np.float16)
    wt = (w * SQRT_PI_2).astype(np.float16)
    xT = np.ascontiguousarray(x.T.reshape(KC, P, B)).astype(np.float32)
    oh = np.broadcast_to(np.eye(B, dtype=np.float16), (P, B, B))
    oh = np.ascontiguousarray(oh)

    in_maps = []
    for d in range(NCORES):
        sl = slice(d * OS, (d + 1) * OS)
        in_maps.append({
            "c": np.ascontiguousarray(c[:, sl].reshape(KC, P, OS, N)),
            "r": np.ascontiguousarray(rinv[:, sl].reshape(KC, P, OS, N)),
            "a": np.ascontiguousarray(A3[:, sl].reshape(KC, P, OS, N)),
            "w": np.ascontiguousarray(wt[:, sl].reshape(KC, P, OS, N)),
            "x": xT,
            "oh": oh,
        })
    return in_maps


def kernel(x, mx_train, scale, sigma, alpha, w, mx_start, _trace=False):
    global LAST_RESULTS
    from concourse.bass_utils import run_bass_kernel_spmd

    if "nc" not in _CACHE:
        _CACHE["nc"] = _build_nc()
    nc = _CACHE["nc"]
    in_maps = _prep_inputs(
        np.asarray(x, np.float32), np.asarray(mx_train, np.float32),
        np.asarray(scale, np.float32), np.asarray(sigma, np.float32),
        np.asarray(alpha, np.float32), np.asarray(w, np.float32),
        np.asarray(mx_start, np.float32),
    )
    res = run_bass_kernel_spmd(nc, in_maps, core_ids=list(range(NCORES)),
                               trace=_trace)
    LAST_RESULTS = res
    return np.concatenate([r["out"] for r in res.results], axis=1)


# revision 5
# speedup vs baseline: 1.4600x; 1.4600x over previous
"""Trainium2 Bass kernel for nn_KATLayer (KAT basis-function layer).

out[b,o] = sum_{i,n} exp(-z^2) * (1 + erf(alpha*z/sqrt(2))) * w[i,o,n]
  z = (x[b,i] - c[i,o,n]) / (|sigma|+1e-8),  c = |scale|*mx_start + mx_train

Sharding: output dim O split across 8 cores (O_shard=64). Per core:
  partitions = i (4 chunks of 128), free = (o_local, n) = 1024 per tile,
  tiles processed in PAIRS (2 consecutive b, same i-chunk) so the
  elementwise/activation ops run at free=2048 to amortize fixed overheads.

Math restructure (all intermediates fp16; validated ~5e-4 rel err vs the
2e-2 gate):
  D  = c - x            [the only cancellation-sensitive op: runs in fp32
                         internally, fp16 out. Split between DVE
                         (tensor_scalar, 2x_2p mode) and ACT (Identity with
                         per-partition bias = -x; Identity is in EVERY act
                         table set so it never forces a table switch)]
  DVE:  zm = D*rinv16   [fp16 TT 2x; = -z]
  ACT:  e  = Derivative_Erf(zm)   [= 2/sqrt(pi)*exp(-z^2), even in z]
  DVE:  q  = e*wt       [wt = w*sqrt(pi)/2 -> q = w*exp(-z^2)]
  DVE:  um = D*A4       [A4 = alpha*rinv/sqrt(2); = -alpha*z/sqrt(2)]
  ACT:  t  = Erf(-um)   [= erf(alpha*z/sqrt(2))]
  DVE:  r  = q*t
  PE :  psum += onehot_b.T @ q ; psum += onehot_b.T @ r
        (the "+1" of (1+erf) is absorbed by accumulating BOTH q and r
        streams in PSUM — no fp16 STT, which only has a 1x uop)
Final: DVE reduce over n: psum(32,64,16) -> (32,64); DMA out.

Per-k consts (rinv16, A4, wt) are read through stride-0 broadcast APs
across the pair dim. Derivative_Erf and Erf live in different ACT table
sets (~2.7us/switch), so pairs are processed in groups with phase-batched
activations (2 switches per group).
"""
import sys

sys.path.insert(0, "/opt/trn_rl_repo")
import math

import numpy as np

B, I, O, N = 32, 512, 512, 16
NCORES = 8
OS = O // NCORES          # 64 output dims per core
KC = I // 128             # 4 i-chunks
P = 128
GQ = 6                    # pairs per activation-phase group (12 tiles)
ACT_D_MOD, ACT_D_LIM = 12, 5   # 5/12 of D ops run on ACT, rest on DVE
INV_SQRT2 = 0.7071067811865476
SQRT_PI_2 = math.sqrt(math.pi) / 2.0

_CACHE = {}
LAST_RESULTS = None


def _build_nc(reps=1, GQ=GQ):
    import concourse.bacc as bacc
    import concourse.mybir as mybir
    from concourse import tile

    fp32 = mybir.dt.float32
    fp16 = mybir.dt.float16
    AF = mybir.ActivationFunctionType
    ALU = mybir.AluOpType

    nc = bacc.Bacc(
        "TRN2", target_bir_lowering=False, debug=False, num_devices=NCORES
    )
    c_d = nc.dram_tensor("c", [KC, P, OS, N], fp32, kind="ExternalInput")
    r_d = nc.dram_tensor("r", [KC, P, OS, N], fp16, kind="ExternalInput")
    a_d = nc.dram_tensor("a", [KC, P, OS, N], fp16, kind="ExternalInput")
    w_d = nc.dram_tensor("w", [KC, P, OS, N], fp16, kind="ExternalInput")
    x_d = nc.dram_tensor("x", [KC, P, B], fp32, kind="ExternalInput")
    xn_d = nc.dram_tensor("xn", [KC, P, B], fp32, kind="ExternalInput")
    oh_d = nc.dram_tensor("oh", [P, B, B], fp16, kind="ExternalInput")
    out_d = nc.dram_tensor("out", [B, OS], fp32, kind="ExternalOutput")

    with tile.TileContext(nc) as tc:
        with (
            tc.tile_pool(name="const", bufs=1) as cpool,
            tc.tile_pool(name="dp", bufs=GQ + 2) as dpool,
            tc.tile_pool(name="qp", bufs=GQ + 2) as qpool,
            tc.tile_pool(name="tp", bufs=GQ + 2) as tpool,
            tc.tile_pool(name="zp", bufs=3) as zpool,
            tc.tile_pool(name="ep", bufs=3) as epool,
            tc.tile_pool(name="up", bufs=3) as upool,
            tc.tile_pool(name="rp", bufs=3) as rpool,
            tc.tile_pool(name="psum", bufs=1, space="PSUM") as psp,
            tc.tile_pool(name="outp", bufs=1) as opool,
        ):
            c_sb, r_sb, a_sb, w_sb = [], [], [], []
            for k in range(KC):
                for lst, dram, nm, dt_ in (
                    (c_sb, c_d, "c", fp32),
                    (r_sb, r_d, "r", fp16),
                    (a_sb, a_d, "a", fp16),
                    (w_sb, w_d, "w", fp16),
                ):
                    t = cpool.tile([P, OS, N], dt_, tag=f"{nm}{k}")
                    nc.sync.dma_start(t[:], dram[k])
                    lst.append(t)
            x_sb = cpool.tile([P, KC * B], fp32, tag="x")
            xn_sb = cpool.tile([P, KC * B], fp32, tag="xn")
            for k in range(KC):
                nc.sync.dma_start(x_sb[:, k * B : (k + 1) * B], x_d[k])
                nc.sync.dma_start(xn_sb[:, k * B : (k + 1) * B], xn_d[k])
            oh_sb = cpool.tile([P, B, B], fp16, tag="oh")
            nc.sync.dma_start(oh_sb[:], oh_d[:])

            psum_t = psp.tile([B, OS, N], fp32)
            # pairs: 2 consecutive b, same k
            pairs = [(k, b) for k in range(KC) for b in range(0, B, 2)]
            n_pairs = len(pairs)
            out_sb = opool.tile([B, OS], fp32)

            def bcast2(t):
                return t[:, None].broadcast_to((P, 2, OS, N))

            for rep in range(reps):
                n_mm = 0
                n_d = 0
                total_mm = 8 * n_pairs
                for g0 in range(0, n_pairs, GQ):
                    grp = pairs[g0 : g0 + GQ]
                    # phase 1: D = c - x (fp32 internal, fp16 out), split
                    # between DVE tensor_scalar (2x_2p) and ACT Identity
                    # (per-partition bias = -x; table-agnostic)
                    ds = []
                    for k, b in grp:
                        d = dpool.tile([P, 2, OS, N], fp16, tag="dp")
                        for j in range(2):
                            col = slice(k * B + b + j, k * B + b + j + 1)
                            if n_d % ACT_D_MOD < ACT_D_LIM:
                                nc.scalar.activation(
                                    d[:, j], c_sb[k][:], AF.Identity,
                                    bias=xn_sb[:, col], scale=1.0,
                                )
                            else:
                                nc.vector.tensor_scalar(
                                    d[:, j], c_sb[k][:], x_sb[:, col], None,
                                    op0=ALU.subtract,
                                )
                            n_d += 1
                        ds.append(d)
                    # phase 2: zm = D*rinv (DVE); e = D_ERF(zm) (ACT table A);
                    #          q = e*wt (DVE)
                    qs = []
                    for (k, b), d in zip(grp, ds):
                        zm = zpool.tile([P, 2, OS, N], fp16)
                        nc.vector.tensor_mul(zm[:], d[:], bcast2(r_sb[k]))
                        e = epool.tile([P, 2, OS, N], fp16)
                        nc.scalar.activation(e[:], zm[:], AF.Derivative_Erf)
                        q = qpool.tile([P, 2, OS, N], fp16, tag="qp")
                        nc.vector.tensor_mul(q[:], e[:], bcast2(w_sb[k]))
                        qs.append(q)
                    # phase 3: um = D*A4 (DVE); t = Erf(-um) (ACT, table B)
                    ts_ = []
                    for (k, b), d in zip(grp, ds):
                        um = upool.tile([P, 2, OS, N], fp16)
                        nc.vector.tensor_mul(um[:], d[:], bcast2(a_sb[k]))
                        t_ = tpool.tile([P, 2, OS, N], fp16, tag="tp")
                        nc.scalar.activation(t_[:], um[:], AF.Erf, scale=-1.0)
                        ts_.append(t_)
                    # phase 4: r = q*t (DVE); accumulate q and r streams (PE)
                    for (k, b), q, t_ in zip(grp, qs, ts_):
                        r_ = rpool.tile([P, 2, OS, N], fp16)
                        nc.vector.tensor_mul(r_[:], q[:], t_[:])
                        for j in range(2):
                            for src in (q, r_):
                                for h in range(2):
                                    nc.tensor.matmul(
                                        psum_t[:, 32 * h : 32 * (h + 1), :],
                                        oh_sb[:, b + j, :],
                                        src[:, j, 32 * h : 32 * (h + 1), :],
                                        start=(n_mm < 2),
                                        stop=(n_mm >= total_mm - 2),
                                    )
                                    n_mm += 1

                nc.vector.tensor_reduce(
                    out_sb[:], psum_t[:], axis=mybir.AxisListType.X, op=ALU.add
                )
            nc.sync.dma_start(out_d[:], out_sb[:])

    nc.compile()
    return nc


def _prep_inputs(x, mx_train, scale, sigma, alpha, w, mx_start):
    c = (np.abs(scale)[:, :, None] * mx_start[None, None, :]
         + mx_train[:, :, None]).astype(np.float32)
    rinv = (1.0 / (np.abs(sigma) + 1e-8)).astype(np.float32)
    r16 = rinv.astype(np.float16)
    A4 = (alpha * rinv * INV_SQRT2).astype(np.float16)
    wt = (w * SQRT_PI_2).astype(np.float16)
    xT = np.ascontiguousarray(x.T.reshape(KC, P, B)).astype(np.float32)
    oh = np.broadcast_to(np.eye(B, dtype=np.float16), (P, B, B))
    oh = np.ascontiguousarray(oh)

    in_maps = []
    for d in range(NCORES):
        sl = slice(d * OS, (d + 1) * OS)
        in_maps.append({
            "c": np.ascontiguousarray(c[:, sl].reshape(KC, P, OS, N)),
            "r": np.ascontiguousarray(r16[:, sl].reshape(KC, P, OS, N)),
            "a": np.ascontiguousarray(A4[:, sl].reshape(KC, P, OS, N)),
            "w": np.ascontiguousarray(wt[:, sl].reshape(KC, P, OS, N)),
            "x": xT,
            "xn": -xT,
            "oh": oh,
        })
    return in_maps


def kernel(x, mx_train, scale, sigma, alpha, w, mx_start, _trace=False):
    global LAST_RESULTS
    from concourse.bass_utils import run_bass_kernel_spmd

    if "nc" not in _CACHE:
        _CACHE["nc"] = _build_nc()
    nc = _CACHE["nc"]
    in_maps = _prep_inputs(
        np.asarray(x, np.float32), np.asarray(mx_train, np.float32),
        np.asarray(scale, np.float32), np.asarray(sigma, np.float32),
        np.asarray(alpha, np.float32), np.asarray(w, np.float32),
        np.asarray(mx_start, np.float32),
    )
    res = run_bass_kernel_spmd(nc, in_maps, core_ids=list(range(NCORES)),
                               trace=_trace)
    LAST_RESULTS = res
    return np.concatenate([r["out"] for r in res.results], axis=1)


# revision 8
# speedup vs baseline: 1.5350x; 1.0514x over previous
"""Trainium2 Bass kernel for nn_KATLayer (KAT basis-function layer).

out[b,o] = sum_{i,n} exp(-z^2) * (1 + erf(alpha*z/sqrt(2))) * w[i,o,n]
  z = (x[b,i] - c[i,o,n]) / (|sigma|+1e-8),  c = |scale|*mx_start + mx_train

Sharding: output dim O split across 8 cores (O_shard=64). Per core:
  partitions = i (4 chunks of 128), free = (o_local, n) = 1024 per tile,
  tiles processed in PAIRS (2 consecutive b, same i-chunk) so the
  elementwise/activation ops run at free=2048 to amortize fixed overheads.

Math restructure (all intermediates fp16; validated ~5e-4 rel err vs the
2e-2 gate):
  D  = c - x            [the only cancellation-sensitive op: runs in fp32
                         internally, fp16 out. Split between DVE
                         (tensor_scalar, 2x_2p mode) and ACT (Identity with
                         per-partition bias = -x; Identity is in EVERY act
                         table set so it never forces a table switch)]
  DVE:  zm = D*rinv16   [fp16 TT 2x; = -z]
  ACT:  e  = Derivative_Erf(zm)   [= 2/sqrt(pi)*exp(-z^2), even in z]
  DVE:  q  = e*wt       [wt = w*sqrt(pi)/2 -> q = w*exp(-z^2)]
  DVE:  um = D*A4       [A4 = alpha*rinv/sqrt(2); = -alpha*z/sqrt(2)]
  ACT:  t  = Erf(-um)   [= erf(alpha*z/sqrt(2))]
  DVE:  r  = q*t
  PE :  psum += onehot_b.T @ q ; psum += onehot_b.T @ r
        (the "+1" of (1+erf) is absorbed by accumulating BOTH q and r
        streams in PSUM — no fp16 STT, which only has a 1x uop)
Final: DVE reduce over n: psum(32,64,16) -> (32,64); DMA out.

Per-k consts (rinv16, A4, wt) are read through stride-0 broadcast APs
across the pair dim. Derivative_Erf and Erf live in different ACT table
sets (~2.7us/switch), so pairs are processed in groups with phase-batched
activations (2 switches per group).
"""
import sys

sys.path.insert(0, "/opt/trn_rl_repo")
import math

import numpy as np

B, I, O, N = 32, 512, 512, 16
NCORES = 8
OS = O // NCORES          # 64 output dims per core
KC = I // 128             # 4 i-chunks
P = 128
GQ = 6                    # pairs per activation-phase group (12 tiles)
ACT_D_MOD, ACT_D_LIM = 12, 5   # 5/12 of D ops run on ACT, rest on DVE
INV_SQRT2 = 0.7071067811865476
SQRT_PI_2 = math.sqrt(math.pi) / 2.0

_CACHE = {}
LAST_RESULTS = None


def _build_nc(reps=1, GQ=GQ):
    import concourse.bacc as bacc
    import concourse.mybir as mybir
    from concourse import tile
    from concourse.tile_rust import add_dep_helper

    fp32 = mybir.dt.float32
    fp16 = mybir.dt.float16
    AF = mybir.ActivationFunctionType
    ALU = mybir.AluOpType

    nc = bacc.Bacc(
        "TRN2", target_bir_lowering=False, debug=False, num_devices=NCORES
    )
    c_d = nc.dram_tensor("c", [KC, P, OS, N], fp32, kind="ExternalInput")
    r_d = nc.dram_tensor("r", [KC, P, OS, N], fp16, kind="ExternalInput")
    a_d = nc.dram_tensor("a", [KC, P, OS, N], fp16, kind="ExternalInput")
    w_d = nc.dram_tensor("w", [KC, P, OS, N], fp16, kind="ExternalInput")
    x_d = nc.dram_tensor("x", [KC, P, B], fp32, kind="ExternalInput")
    xn_d = nc.dram_tensor("xn", [KC, P, B], fp32, kind="ExternalInput")
    oh_d = nc.dram_tensor("oh", [P, B, B], fp16, kind="ExternalInput")
    out_d = nc.dram_tensor("out", [B, OS], fp32, kind="ExternalOutput")

    with tile.TileContext(nc) as tc:
        with (
            tc.tile_pool(name="const", bufs=1) as cpool,
            tc.tile_pool(name="dp", bufs=GQ + 2) as dpool,
            tc.tile_pool(name="qp", bufs=GQ + 2) as qpool,
            tc.tile_pool(name="tp", bufs=GQ + 2) as tpool,
            tc.tile_pool(name="zp", bufs=3) as zpool,
            tc.tile_pool(name="ep", bufs=3) as epool,
            tc.tile_pool(name="up", bufs=3) as upool,
            tc.tile_pool(name="rp", bufs=3) as rpool,
            tc.tile_pool(name="psum", bufs=1, space="PSUM") as psp,
            tc.tile_pool(name="outp", bufs=1) as opool,
        ):
            c_sb, r_sb, a_sb, w_sb = [], [], [], []
            for k in range(KC):
                for lst, dram, nm, dt_ in (
                    (c_sb, c_d, "c", fp32),
                    (r_sb, r_d, "r", fp16),
                    (a_sb, a_d, "a", fp16),
                    (w_sb, w_d, "w", fp16),
                ):
                    t = cpool.tile([P, OS, N], dt_, tag=f"{nm}{k}")
                    nc.sync.dma_start(t[:], dram[k])
                    lst.append(t)
            x_sb = cpool.tile([P, KC * B], fp32, tag="x")
            xn_sb = cpool.tile([P, KC * B], fp32, tag="xn")
            for k in range(KC):
                nc.sync.dma_start(x_sb[:, k * B : (k + 1) * B], x_d[k])
                nc.sync.dma_start(xn_sb[:, k * B : (k + 1) * B], xn_d[k])
            oh_sb = cpool.tile([P, B, B], fp16, tag="oh")
            nc.sync.dma_start(oh_sb[:], oh_d[:])

            psum_t = psp.tile([B, OS, N], fp32)
            # pairs: 2 consecutive b, same k
            pairs = [(k, b) for k in range(KC) for b in range(0, B, 2)]
            n_pairs = len(pairs)
            out_sb = opool.tile([B, OS], fp32)

            def bcast2(t):
                return t[:, None].broadcast_to((P, 2, OS, N))

            for rep in range(reps):
                n_mm = 0
                n_d = 0
                total_mm = 8 * n_pairs
                prev_erf = None
                for g0 in range(0, n_pairs, GQ):
                    grp = pairs[g0 : g0 + GQ]
                    # phase 1: D = c - x (fp32 internal, fp16 out), split
                    # between DVE tensor_scalar (2x_2p) and ACT Identity
                    # (per-partition bias = -x; table-agnostic)
                    ds = []
                    for k, b in grp:
                        d = dpool.tile([P, 2, OS, N], fp16, tag="dp")
                        for j in range(2):
                            col = slice(k * B + b + j, k * B + b + j + 1)
                            if n_d % ACT_D_MOD < ACT_D_LIM:
                                nc.scalar.activation(
                                    d[:, j], c_sb[k][:], AF.Identity,
                                    bias=xn_sb[:, col], scale=1.0,
                                )
                            else:
                                nc.vector.tensor_scalar(
                                    d[:, j], c_sb[k][:], x_sb[:, col], None,
                                    op0=ALU.subtract,
                                )
                            n_d += 1
                        ds.append(d)
                    # phase 2: zm = D*rinv (DVE); e = D_ERF(zm) (ACT table A);
                    #          q = e*wt (DVE)
                    qs = []
                    derfs = []
                    for (k, b), d in zip(grp, ds):
                        zm = zpool.tile([P, 2, OS, N], fp16)
                        nc.vector.tensor_mul(zm[:], d[:], bcast2(r_sb[k]))
                        e = epool.tile([P, 2, OS, N], fp16)
                        ei = nc.scalar.activation(e[:], zm[:], AF.Derivative_Erf)
                        # keep ACT table phases clean: no D_ERF may be
                        # scheduled before the previous group's Erf run ends
                        if prev_erf is not None:
                            add_dep_helper(ei.ins, prev_erf.ins, sync=False,
                                           reason="act table phase order")
                        derfs.append(ei)
                        q = qpool.tile([P, 2, OS, N], fp16, tag="qp")
                        nc.vector.tensor_mul(q[:], e[:], bcast2(w_sb[k]))
                        qs.append(q)
                    # phase 3: um = D*A4 (DVE); t = Erf(-um) (ACT, table B)
                    ts_ = []
                    for (k, b), d in zip(grp, ds):
                        um = upool.tile([P, 2, OS, N], fp16)
                        nc.vector.tensor_mul(um[:], d[:], bcast2(a_sb[k]))
                        t_ = tpool.tile([P, 2, OS, N], fp16, tag="tp")
                        ti = nc.scalar.activation(t_[:], um[:], AF.Erf,
                                                  scale=-1.0)
                        # no Erf before this group's D_ERF run ends
                        add_dep_helper(ti.ins, derfs[-1].ins, sync=False,
                                       reason="act table phase order")
                        prev_erf = ti
                        ts_.append(t_)
                    # phase 4: r = q*t (DVE); accumulate q and r streams (PE)
                    for (k, b), q, t_ in zip(grp, qs, ts_):
                        r_ = rpool.tile([P, 2, OS, N], fp16)
                        nc.vector.tensor_mul(r_[:], q[:], t_[:])
                        for j in range(2):
                            for src in (q, r_):
                                for h in range(2):
                                    nc.tensor.matmul(
                                        psum_t[:, 32 * h : 32 * (h + 1), :],
                                        oh_sb[:, b + j, :],
                                        src[:, j, 32 * h : 32 * (h + 1), :],
                                        start=(n_mm < 2),
                                        stop=(n_mm >= total_mm - 2),
                                    )
                                    n_mm += 1

                nc.vector.tensor_reduce(
                    out_sb[:], psum_t[:], axis=mybir.AxisListType.X, op=ALU.add
                )
            nc.sync.dma_start(out_d[:], out_sb[:])

    nc.compile()
    return nc


def _prep_inputs(x, mx_train, scale, sigma, alpha, w, mx_start):
    c = (np.abs(scale)[:, :, None] * mx_start[None, None, :]
         + mx_train[:, :, None]).astype(np.float32)
    rinv = (1.0 / (np.abs(sigma) + 1e-8)).astype(np.float32)
    r16 = rinv.astype(np.float16)
    A4 = (alpha * rinv * INV_SQRT2).astype(np.float16)
    wt = (w * SQRT_PI_2).astype(np.float16)
    xT = np.ascontiguousarray(x.T.reshape(KC, P, B)).astype(np.float32)
    oh = np.broadcast_to(np.eye(B, dtype=np.float16), (P, B, B))
    oh = np.ascontiguousarray(oh)

    in_maps = []
    for d in range(NCORES):
        sl = slice(d * OS, (d + 1) * OS)
        in_maps.append({
            "c": np.ascontiguousarray(c[:, sl].reshape(KC, P, OS, N)),
            "r": np.ascontiguousarray(r16[:, sl].reshape(KC, P, OS, N)),
            "a": np.ascontiguousarray(A4[:, sl].reshape(KC, P, OS, N)),
            "w": np.ascontiguousarray(wt[:, sl].reshape(KC, P, OS, N)),
            "x": xT,
            "xn": -xT,
            "oh": oh,
        })
    return in_maps


def kernel(x, mx_train, scale, sigma, alpha, w, mx_start, _trace=False):
    global LAST_RESULTS
    from concourse.bass_utils import run_bass_kernel_spmd

    if "nc" not in _CACHE:
        _CACHE["nc"] = _build_nc()
    nc = _CACHE["nc"]
    in_maps = _prep_inputs(
        np.asarray(x, np.float32), np.asarray(mx_train, np.float32),
        np.asarray(scale, np.float32), np.asarray(sigma, np.float32),
        np.asarray(alpha, np.float32), np.asarray(w, np.float32),
        np.asarray(mx_start, np.float32),
    )
    res = run_bass_kernel_spmd(nc, in_maps, core_ids=list(range(NCORES)),
                               trace=_trace)
    LAST_RESULTS = res
    return np.concatenate([r["out"] for r in res.results], axis=1)


# revision 13
# speedup vs baseline: 1.5384x; 1.0022x over previous
"""Trainium2 Bass kernel for nn_KATLayer (KAT basis-function layer).

out[b,o] = sum_{i,n} exp(-z^2) * (1 + erf(alpha*z/sqrt(2))) * w[i,o,n]
  z = (x[b,i] - c[i,o,n]) / (|sigma|+1e-8),  c = |scale|*mx_start + mx_train

Sharding: output dim O split across 8 cores (O_shard=64). Per core:
  partitions = i (4 chunks of 128), free = (o_local, n) = 1024 per tile,
  tiles processed in PAIRS (2 consecutive b, same i-chunk) so the
  elementwise/activation ops run at free=2048 to amortize fixed overheads.

Math restructure (all intermediates fp16; validated ~5e-4 rel err vs the
2e-2 gate):
  D  = c - x            [the only cancellation-sensitive op: runs in fp32
                         internally, fp16 out. Split between DVE
                         (tensor_scalar, 2x_2p mode) and ACT (Identity with
                         per-partition bias = -x; Identity is in EVERY act
                         table set so it never forces a table switch)]
  DVE:  zm = D*rinv16   [fp16 TT 2x; = -z]
  ACT:  e  = Derivative_Erf(zm)   [= 2/sqrt(pi)*exp(-z^2), even in z]
  DVE:  q  = e*wt       [wt = w*sqrt(pi)/2 -> q = w*exp(-z^2)]
  DVE:  um = D*A4       [A4 = alpha*rinv/sqrt(2); = -alpha*z/sqrt(2)]
  ACT:  t  = Erf(-um)   [= erf(alpha*z/sqrt(2))]
  DVE:  r  = q*t
  PE :  psum += onehot_b.T @ q ; psum += onehot_b.T @ r
        (the "+1" of (1+erf) is absorbed by accumulating BOTH q and r
        streams in PSUM — no fp16 STT, which only has a 1x uop)
Final: DVE reduce over n: psum(32,64,16) -> (32,64); DMA out.

Per-k consts (rinv16, A4, wt) are read through stride-0 broadcast APs
across the pair dim. Derivative_Erf and Erf live in different ACT table
sets (~2.7us/switch), so pairs are processed in groups with phase-batched
activations (2 switches per group).
"""
import sys

sys.path.insert(0, "/opt/trn_rl_repo")
import math

import numpy as np

B, I, O, N = 32, 512, 512, 16
NCORES = 8
OS = O // NCORES          # 64 output dims per core
KC = I // 128             # 4 i-chunks
P = 128
GQ = 7                    # pairs per activation-phase group (14 tiles)
ACT_D_MOD, ACT_D_LIM = 2, 1    # 1/2 of D ops run on ACT, rest on DVE
INV_SQRT2 = 0.7071067811865476
SQRT_PI_2 = math.sqrt(math.pi) / 2.0

_CACHE = {}
LAST_RESULTS = None


def _build_nc(reps=1, GQ=GQ):
    import concourse.bacc as bacc
    import concourse.mybir as mybir
    from concourse import tile
    from concourse.tile_rust import add_dep_helper

    fp32 = mybir.dt.float32
    fp16 = mybir.dt.float16
    AF = mybir.ActivationFunctionType
    ALU = mybir.AluOpType

    nc = bacc.Bacc(
        "TRN2", target_bir_lowering=False, debug=False, num_devices=NCORES
    )
    c_d = nc.dram_tensor("c", [KC, P, OS, N], fp32, kind="ExternalInput")
    r_d = nc.dram_tensor("r", [KC, P, OS, N], fp16, kind="ExternalInput")
    a_d = nc.dram_tensor("a", [KC, P, OS, N], fp16, kind="ExternalInput")
    w_d = nc.dram_tensor("w", [KC, P, OS, N], fp16, kind="ExternalInput")
    x_d = nc.dram_tensor("x", [KC, P, B], fp32, kind="ExternalInput")
    xn_d = nc.dram_tensor("xn", [KC, P, B], fp32, kind="ExternalInput")
    oh_d = nc.dram_tensor("oh", [P, B, B], fp16, kind="ExternalInput")
    out_d = nc.dram_tensor("out", [B, OS], fp32, kind="ExternalOutput")

    with tile.TileContext(nc) as tc:
        with (
            tc.tile_pool(name="const", bufs=1) as cpool,
            tc.tile_pool(name="dp", bufs=GQ + 2) as dpool,
            tc.tile_pool(name="qp", bufs=GQ + 2) as qpool,
            tc.tile_pool(name="tp", bufs=GQ + 2) as tpool,
            tc.tile_pool(name="zp", bufs=2) as zpool,
            tc.tile_pool(name="ep", bufs=2) as epool,
            tc.tile_pool(name="up", bufs=2) as upool,
            tc.tile_pool(name="rp", bufs=2) as rpool,
            tc.tile_pool(name="psum", bufs=1, space="PSUM") as psp,
            tc.tile_pool(name="outp", bufs=1) as opool,
        ):
            c_sb, r_sb, a_sb, w_sb = [], [], [], []
            for k in range(KC):
                for lst, dram, nm, dt_ in (
                    (c_sb, c_d, "c", fp32),
                    (r_sb, r_d, "r", fp16),
                    (a_sb, a_d, "a", fp16),
                    (w_sb, w_d, "w", fp16),
                ):
                    t = cpool.tile([P, OS, N], dt_, tag=f"{nm}{k}")
                    nc.sync.dma_start(t[:], dram[k])
                    lst.append(t)
            x_sb = cpool.tile([P, KC * B], fp32, tag="x")
            xn_sb = cpool.tile([P, KC * B], fp32, tag="xn")
            for k in range(KC):
                nc.sync.dma_start(x_sb[:, k * B : (k + 1) * B], x_d[k])
                nc.sync.dma_start(xn_sb[:, k * B : (k + 1) * B], xn_d[k])
            oh_sb = cpool.tile([P, B, B], fp16, tag="oh")
            nc.sync.dma_start(oh_sb[:], oh_d[:])

            psum_t = psp.tile([B, OS, N], fp32)
            # pairs: 2 consecutive b, same k
            pairs = [(k, b) for k in range(KC) for b in range(0, B, 2)]
            n_pairs = len(pairs)
            out_sb = opool.tile([B, OS], fp32)

            def bcast2(t):
                return t[:, None].broadcast_to((P, 2, OS, N))

            for rep in range(reps):
                n_mm = 0
                n_d = 0
                total_mm = 8 * n_pairs
                prev_erf = None
                # taper the final groups so the wind-down pipeline is short
                sizes = []
                left = n_pairs
                while left > GQ + 4:
                    sizes.append(GQ)
                    left -= GQ
                while left > 0:
                    s = max(2, min(left, (left + 1) // 2))
                    sizes.append(s)
                    left -= s
                bounds = []
                g0 = 0
                for s in sizes:
                    bounds.append((g0, g0 + s))
                    g0 += s
                for g0, g1 in bounds:
                    grp = pairs[g0:g1]
                    # phase 1: D = c - x (fp32 internal, fp16 out), split
                    # between DVE tensor_scalar (2x_2p) and ACT Identity
                    # (per-partition bias = -x; table-agnostic)
                    ds = []
                    for k, b in grp:
                        d = dpool.tile([P, 2, OS, N], fp16, tag="dp")
                        for j in range(2):
                            col = slice(k * B + b + j, k * B + b + j + 1)
                            if n_d % ACT_D_MOD < ACT_D_LIM:
                                nc.scalar.activation(
                                    d[:, j], c_sb[k][:], AF.Identity,
                                    bias=xn_sb[:, col], scale=1.0,
                                )
                            else:
                                nc.vector.tensor_scalar(
                                    d[:, j], c_sb[k][:], x_sb[:, col], None,
                                    op0=ALU.subtract,
                                )
                            n_d += 1
                        ds.append(d)
                    # phase 2: zm = D*rinv (DVE); e = D_ERF(zm) (ACT table A);
                    #          q = e*wt (DVE)
                    qs = []
                    derfs = []
                    for (k, b), d in zip(grp, ds):
                        zm = zpool.tile([P, 2, OS, N], fp16)
                        nc.vector.tensor_mul(zm[:], d[:], bcast2(r_sb[k]))
                        e = epool.tile([P, 2, OS, N], fp16)
                        ei = nc.scalar.activation(e[:], zm[:], AF.Derivative_Erf)
                        # keep ACT table phases clean: no D_ERF may be
                        # scheduled before the previous group's Erf run ends
                        if prev_erf is not None:
                            add_dep_helper(ei.ins, prev_erf.ins, sync=False,
                                           reason="act table phase order")
                        derfs.append(ei)
                        q = qpool.tile([P, 2, OS, N], fp16, tag="qp")
                        nc.vector.tensor_mul(q[:], e[:], bcast2(w_sb[k]))
                        qs.append(q)
                        # q-stream matmuls issue here (phase 2) to spread PE
                        # load and shrink the end-of-kernel drain
                        for j in range(2):
                            for h in range(2):
                                nc.tensor.matmul(
                                    psum_t[:, 32 * h : 32 * (h + 1), :],
                                    oh_sb[:, b + j, :],
                                    q[:, j, 32 * h : 32 * (h + 1), :],
                                    start=(n_mm < 2),
                                    stop=(n_mm >= total_mm - 2),
                                )
                                n_mm += 1
                    # phase 3: um = D*A4 (DVE); t = Erf(-um) (ACT, table B)
                    ts_ = []
                    for (k, b), d in zip(grp, ds):
                        um = upool.tile([P, 2, OS, N], fp16)
                        nc.vector.tensor_mul(um[:], d[:], bcast2(a_sb[k]))
                        t_ = tpool.tile([P, 2, OS, N], fp16, tag="tp")
                        ti = nc.scalar.activation(t_[:], um[:], AF.Erf,
                                                  scale=-1.0)
                        # no Erf before this group's D_ERF run ends
                        add_dep_helper(ti.ins, derfs[-1].ins, sync=False,
                                       reason="act table phase order")
                        prev_erf = ti
                        ts_.append(t_)
                    # phase 4: r = q*t (DVE); accumulate r stream (PE)
                    for (k, b), q, t_ in zip(grp, qs, ts_):
                        r_ = rpool.tile([P, 2, OS, N], fp16)
                        nc.vector.tensor_mul(r_[:], q[:], t_[:])
                        for j in range(2):
                            for h in range(2):
                                nc.tensor.matmul(
                                    psum_t[:, 32 * h : 32 * (h + 1), :],
                                    oh_sb[:, b + j, :],
                                    r_[:, j, 32 * h : 32 * (h + 1), :],
                                    start=(n_mm < 2),
                                    stop=(n_mm >= total_mm - 2),
                                )
                                n_mm += 1

                nc.vector.tensor_reduce(
                    out_sb[:], psum_t[:], axis=mybir.AxisListType.X, op=ALU.add
                )
            nc.sync.dma_start(out_d[:], out_sb[:])

    nc.compile()
    return nc


def _prep_inputs(x, mx_train, scale, sigma, alpha, w, mx_start):
    c = (np.abs(scale)[:, :, None] * mx_start[None, None, :]
         + mx_train[:, :, None]).astype(np.float32)
    rinv = (1.0 / (np.abs(sigma) + 1e-8)).astype(np.float32)
    r16 = rinv.astype(np.float16)
    A4 = (alpha * rinv * INV_SQRT2).astype(np.float16)
    wt = (w * SQRT_PI_2).astype(np.float16)
    xT = np.ascontiguousarray(x.T.reshape(KC, P, B)).astype(np.float32)
    oh = np.broadcast_to(np.eye(B, dtype=np.float16), (P, B, B))
    oh = np.ascontiguousarray(oh)

    in_maps = []
    for d in range(NCORES):
        sl = slice(d * OS, (d + 1) * OS)
        in_maps.append({
            "c": np.ascontiguousarray(c[:, sl].reshape(KC, P, OS, N)),
            "r": np.ascontiguousarray(r16[:, sl].reshape(KC, P, OS, N)),
            "a": np.ascontiguousarray(A4[:, sl].reshape(KC, P, OS, N)),
            "w": np.ascontiguousarray(wt[:, sl].reshape(KC, P, OS, N)),
            "x": xT,
            "xn": -xT,
            "oh": oh,
        })
    return in_maps


def kernel(x, mx_train, scale, sigma, alpha, w, mx_start, _trace=False):
    global LAST_RESULTS
    from concourse.bass_utils import run_bass_kernel_spmd

    if "nc" not in _CACHE:
        _CACHE["nc"] = _build_nc()
    nc = _CACHE["nc"]
    in_maps = _prep_inputs(
        np.asarray(x, np.float32), np.asarray(mx_train, np.float32),
        np.asarray(scale, np.float32), np.asarray(sigma, np.float32),
        np.asarray(alpha, np.float32), np.asarray(w, np.float32),
        np.asarray(mx_start, np.float32),
    )
    res = run_bass_kernel_spmd(nc, in_maps, core_ids=list(range(NCORES)),
                               trace=_trace)
    LAST_RESULTS = res
    return np.concatenate([r["out"] for r in res.results], axis=1)


# revision 17
# speedup vs baseline: 1.5770x; 1.0250x over previous
"""Trainium2 Bass kernel for nn_KATLayer (KAT basis-function layer).

out[b,o] = sum_{i,n} exp(-z^2) * (1 + erf(alpha*z/sqrt(2))) * w[i,o,n]
  z = (x[b,i] - c[i,o,n]) / (|sigma|+1e-8),  c = |scale|*mx_start + mx_train

Sharding: output dim O split across 8 cores (O_shard=64). Per core:
  partitions = i (4 chunks of 128), free = (o_local, n) = 1024 per tile,
  tiles processed in PAIRS (2 consecutive b, same i-chunk) so the
  elementwise/activation ops run at free=2048 to amortize fixed overheads.

Math restructure (all intermediates fp16; validated ~5e-4 rel err vs the
2e-2 gate):
  D  = c - x            [the only cancellation-sensitive op: runs in fp32
                         internally, fp16 out. Split between DVE
                         (tensor_scalar, 2x_2p mode) and ACT (Identity with
                         per-partition bias = -x; Identity is in EVERY act
                         table set so it never forces a table switch)]
  DVE:  zm = D*rinv16   [fp16 TT 2x; = -z]
  ACT:  e  = Derivative_Erf(zm)   [= 2/sqrt(pi)*exp(-z^2), even in z]
  DVE:  q  = e*wt       [wt = w*sqrt(pi)/2 -> q = w*exp(-z^2)]
  DVE:  um = D*A4       [A4 = alpha*rinv/sqrt(2); = -alpha*z/sqrt(2)]
  ACT:  t  = Erf(-um)   [= erf(alpha*z/sqrt(2))]
  DVE:  r  = q*t
  PE :  psum += onehot_b.T @ q ; psum += onehot_b.T @ r
        (the "+1" of (1+erf) is absorbed by accumulating BOTH q and r
        streams in PSUM — no fp16 STT, which only has a 1x uop)
Final: DVE reduce over n: psum(32,64,16) -> (32,64); DMA out.

Per-k consts (rinv16, A4, wt) are read through stride-0 broadcast APs
across the pair dim. Derivative_Erf and Erf live in different ACT table
sets (~2.7us/switch), so pairs are processed in groups with phase-batched
activations (2 switches per group).
"""
import sys

sys.path.insert(0, "/opt/trn_rl_repo")
import math

import numpy as np

B, I, O, N = 32, 512, 512, 16
NCORES = 8
OS = O // NCORES          # 64 output dims per core
KC = I // 128             # 4 i-chunks
P = 128
GQ = 8                    # pairs per activation-phase group (16 tiles)
ACT_D_MOD, ACT_D_LIM = 2, 1    # 1/2 of D ops run on ACT, rest on DVE
INV_SQRT2 = 0.7071067811865476
SQRT_PI_2 = math.sqrt(math.pi) / 2.0

_CACHE = {}
LAST_RESULTS = None


def _build_nc(reps=1, GQ=GQ):
    import concourse.bacc as bacc
    import concourse.mybir as mybir
    from concourse import tile
    from concourse.tile_rust import add_dep_helper

    fp32 = mybir.dt.float32
    fp16 = mybir.dt.float16
    AF = mybir.ActivationFunctionType
    ALU = mybir.AluOpType

    nc = bacc.Bacc(
        "TRN2", target_bir_lowering=False, debug=False, num_devices=NCORES
    )
    c_d = nc.dram_tensor("c", [KC, P, OS, N], fp32, kind="ExternalInput")
    r_d = nc.dram_tensor("r", [KC, P, OS, N], fp16, kind="ExternalInput")
    a_d = nc.dram_tensor("a", [KC, P, OS, N], fp16, kind="ExternalInput")
    w_d = nc.dram_tensor("w", [KC, P, OS, N], fp16, kind="ExternalInput")
    x_d = nc.dram_tensor("x", [KC, P, B], fp32, kind="ExternalInput")
    xn_d = nc.dram_tensor("xn", [KC, P, B], fp32, kind="ExternalInput")
    oh_d = nc.dram_tensor("oh", [P, B, B], fp16, kind="ExternalInput")
    out_d = nc.dram_tensor("out", [B, OS], fp32, kind="ExternalOutput")

    with tile.TileContext(nc) as tc:
        with (
            tc.tile_pool(name="const", bufs=1) as cpool,
            tc.tile_pool(name="dp", bufs=GQ + 2) as dpool,
            tc.tile_pool(name="qp", bufs=GQ + 2) as qpool,
            tc.tile_pool(name="tp", bufs=GQ + 2) as tpool,
            tc.tile_pool(name="zp", bufs=2) as zpool,
            tc.tile_pool(name="ep", bufs=2) as epool,
            tc.tile_pool(name="psum", bufs=1, space="PSUM") as psp,
            tc.tile_pool(name="outp", bufs=1) as opool,
        ):
            # small tensors first so compute can start early; then k-major
            # const chunks in consumption order
            x_sb = cpool.tile([P, KC * B], fp32, tag="x")
            xn_sb = cpool.tile([P, KC * B], fp32, tag="xn")
            oh_sb = cpool.tile([P, B, B], fp16, tag="oh")
            for k in range(KC):
                nc.sync.dma_start(x_sb[:, k * B : (k + 1) * B], x_d[k])
                nc.sync.dma_start(xn_sb[:, k * B : (k + 1) * B], xn_d[k])
            nc.sync.dma_start(oh_sb[:], oh_d[:])
            c_sb, r_sb, a_sb, w_sb = [], [], [], []
            for k in range(KC):
                for lst, dram, nm, dt_ in (
                    (c_sb, c_d, "c", fp32),
                    (r_sb, r_d, "r", fp16),
                    (a_sb, a_d, "a", fp16),
                    (w_sb, w_d, "w", fp16),
                ):
                    t = cpool.tile([P, OS, N], dt_, tag=f"{nm}{k}")
                    nc.sync.dma_start(t[:], dram[k])
                    lst.append(t)

            psum_t = psp.tile([B, OS, N], fp32)
            # pairs: 2 consecutive b, same k
            pairs = [(k, b) for k in range(KC) for b in range(0, B, 2)]
            n_pairs = len(pairs)
            out_sb = opool.tile([B, OS], fp32)

            def bcast2(t):
                return t[:, None].broadcast_to((P, 2, OS, N))

            for rep in range(reps):
                n_mm = 0
                n_d = 0
                total_mm = 8 * n_pairs
                prev_erf = None
                # taper the final groups so the wind-down pipeline is short
                sizes = []
                left = n_pairs
                while left > GQ + 4:
                    sizes.append(GQ)
                    left -= GQ
                while left > 0:
                    s = max(2, min(left, (left + 1) // 2))
                    sizes.append(s)
                    left -= s
                bounds = []
                g0 = 0
                for s in sizes:
                    bounds.append((g0, g0 + s))
                    g0 += s
                for g0, g1 in bounds:
                    grp = pairs[g0:g1]
                    # phase 1: D = c - x (fp32 internal, fp16 out), split
                    # between DVE tensor_scalar (2x_2p) and ACT Identity
                    # (per-partition bias = -x; table-agnostic)
                    ds = []
                    for k, b in grp:
                        d = dpool.tile([P, 2, OS, N], fp16, tag="dp")
                        for j in range(2):
                            col = slice(k * B + b + j, k * B + b + j + 1)
                            if n_d % ACT_D_MOD < ACT_D_LIM:
                                nc.scalar.activation(
                                    d[:, j], c_sb[k][:], AF.Identity,
                                    bias=xn_sb[:, col], scale=1.0,
                                )
                            else:
                                nc.vector.tensor_scalar(
                                    d[:, j], c_sb[k][:], x_sb[:, col], None,
                                    op0=ALU.subtract,
                                )
                            n_d += 1
                        ds.append(d)
                    # phase 2: zm = D*rinv (DVE); e = D_ERF(zm) (ACT table A);
                    #          q = e*wt (DVE)
                    qs = []
                    derfs = []
                    for (k, b), d in zip(grp, ds):
                        zm = zpool.tile([P, 2, OS, N], fp16)
                        nc.vector.tensor_mul(zm[:], d[:], bcast2(r_sb[k]))
                        e = epool.tile([P, 2, OS, N], fp16)
                        ei = nc.scalar.activation(e[:], zm[:], AF.Derivative_Erf)
                        # keep ACT table phases clean: no D_ERF may be
                        # scheduled before the previous group's Erf run ends
                        if prev_erf is not None:
                            add_dep_helper(ei.ins, prev_erf.ins, sync=False,
                                           reason="act table phase order")
                        derfs.append(ei)
                        q = qpool.tile([P, 2, OS, N], fp16, tag="qp")
                        nc.vector.tensor_mul(q[:], e[:], bcast2(w_sb[k]))
                        qs.append(q)
                        # q-stream matmuls issue here (phase 2) to spread PE
                        # load and shrink the end-of-kernel drain
                        for j in range(2):
                            for h in range(2):
                                nc.tensor.matmul(
                                    psum_t[:, 32 * h : 32 * (h + 1), :],
                                    oh_sb[:, b + j, :],
                                    q[:, j, 32 * h : 32 * (h + 1), :],
                                    start=(n_mm < 2),
                                    stop=(n_mm >= total_mm - 2),
                                )
                                n_mm += 1
                    # phase 3: um = D*A4 (DVE); t = Erf(-um) in place (ACT,
                    # table B)
                    ts_ = []
                    for (k, b), d in zip(grp, ds):
                        um = tpool.tile([P, 2, OS, N], fp16, tag="tp")
                        nc.vector.tensor_mul(um[:], d[:], bcast2(a_sb[k]))
                        ti = nc.scalar.activation(um[:], um[:], AF.Erf,
                                                  scale=-1.0)
                        # no Erf before this group's D_ERF run ends
                        add_dep_helper(ti.ins, derfs[-1].ins, sync=False,
                                       reason="act table phase order")
                        prev_erf = ti
                        ts_.append(um)
                    # phase 4: r = q*t overwrites q (DVE, after q's matmuls);
                    # accumulate r stream (PE)
                    for (k, b), q, t_ in zip(grp, qs, ts_):
                        nc.vector.tensor_mul(q[:], q[:], t_[:])
                        for j in range(2):
                            for h in range(2):
                                nc.tensor.matmul(
                                    psum_t[:, 32 * h : 32 * (h + 1), :],
                                    oh_sb[:, b + j, :],
                                    q[:, j, 32 * h : 32 * (h + 1), :],
                                    start=(n_mm < 2),
                                    stop=(n_mm >= total_mm - 2),
                                )
                                n_mm += 1

                nc.vector.tensor_reduce(
                    out_sb[:], psum_t[:], axis=mybir.AxisListType.X, op=ALU.add
                )
            nc.sync.dma_start(out_d[:], out_sb[:])

    nc.compile()
    return nc


def _prep_inputs(x, mx_train, scale, sigma, alpha, w, mx_start):
    c = (np.abs(scale)[:, :, None] * mx_start[None, None, :]
         + mx_train[:, :, None]).astype(np.float32)
    rinv = (1.0 / (np.abs(sigma) + 1e-8)).astype(np.float32)
    r16 = rinv.astype(np.float16)
    A4 = (alpha * rinv * INV_SQRT2).astype(np.float16)
    wt = (w * SQRT_PI_2).astype(np.float16)
    xT = np.ascontiguousarray(x.T.reshape(KC, P, B)).astype(np.float32)
    oh = np.broadcast_to(np.eye(B, dtype=np.float16), (P, B, B))
    oh = np.ascontiguousarray(oh)

    in_maps = []
    for d in range(NCORES):
        sl = slice(d * OS, (d + 1) * OS)
        in_maps.append({
            "c": np.ascontiguousarray(c[:, sl].reshape(KC, P, OS, N)),
            "r": np.ascontiguousarray(r16[:, sl].reshape(KC, P, OS, N)),
            "a": np.ascontiguousarray(A4[:, sl].reshape(KC, P, OS, N)),
            "w": np.ascontiguousarray(wt[:, sl].reshape(KC, P, OS, N)),
            "x": xT,
            "xn": -xT,
            "oh": oh,
        })
    return in_maps


def kernel(x, mx_train, scale, sigma, alpha, w, mx_start, _trace=False):
    global LAST_RESULTS
    from concourse.bass_utils import run_bass_kernel_spmd

    if "nc" not in _CACHE:
        _CACHE["nc"] = _build_nc()
    nc = _CACHE["nc"]
    in_maps = _prep_inputs(
        np.asarray(x, np.float32), np.asarray(mx_train, np.float32),
        np.asarray(scale, np.float32), np.asarray(sigma, np.float32),
        np.asarray(alpha, np.float32), np.asarray(w, np.float32),
        np.asarray(mx_start, np.float32),
    )
    res = run_bass_kernel_spmd(nc, in_maps, core_ids=list(range(NCORES)),
                               trace=_trace)
    LAST_RESULTS = res
    return np.concatenate([r["out"] for r in res.results], axis=1)


# revision 20
# speedup vs baseline: 1.6018x; 1.0158x over previous
"""Trainium2 Bass kernel for nn_KATLayer (KAT basis-function layer).

out[b,o] = sum_{i,n} exp(-z^2) * (1 + erf(alpha*z/sqrt(2))) * w[i,o,n]
  z = (x[b,i] - c[i,o,n]) / (|sigma|+1e-8),  c = |scale|*mx_start + mx_train

Sharding: output dim O split across 8 cores (O_shard=64). Per core:
  partitions = i (4 chunks of 128), free = (o_local, n) = 1024 per tile,
  tiles processed in PAIRS (2 consecutive b, same i-chunk) so the
  elementwise/activation ops run at free=2048 to amortize fixed overheads.

Math restructure (all intermediates fp16; validated ~5e-4 rel err vs the
2e-2 gate):
  D  = c - x            [the only cancellation-sensitive op: runs in fp32
                         internally, fp16 out. Split between DVE
                         (tensor_scalar, 2x_2p mode) and ACT (Identity with
                         per-partition bias = -x; Identity is in EVERY act
                         table set so it never forces a table switch)]
  DVE:  zm = D*rinv16   [fp16 TT 2x; = -z]
  ACT:  e  = Derivative_Erf(zm)   [= 2/sqrt(pi)*exp(-z^2), even in z]
  DVE:  q  = e*wt       [wt = w*sqrt(pi)/2 -> q = w*exp(-z^2)]
  DVE:  um = D*A4       [A4 = alpha*rinv/sqrt(2); = -alpha*z/sqrt(2)]
  ACT:  t  = Erf(-um)   [= erf(alpha*z/sqrt(2))]
  DVE:  r  = q*t
  PE :  psum += onehot_b.T @ q ; psum += onehot_b.T @ r
        (the "+1" of (1+erf) is absorbed by accumulating BOTH q and r
        streams in PSUM — no fp16 STT, which only has a 1x uop)
Final: DVE reduce over n: psum(32,64,16) -> (32,64); DMA out.

Per-k consts (rinv16, A4, wt) are read through stride-0 broadcast APs
across the pair dim. Derivative_Erf and Erf live in different ACT table
sets (~2.7us/switch), so pairs are processed in groups with phase-batched
activations (2 switches per group).
"""
import sys

sys.path.insert(0, "/opt/trn_rl_repo")
import math

import numpy as np

B, I, O, N = 32, 512, 512, 16
NCORES = 8
OS = O // NCORES          # 64 output dims per core
KC = I // 128             # 4 i-chunks
P = 128
GQ = 8                    # pairs per activation-phase group (16 tiles)
ACT_D_MOD, ACT_D_LIM = 2, 1    # 1/2 of D ops run on ACT, rest on DVE
INV_SQRT2 = 0.7071067811865476
SQRT_PI_2 = math.sqrt(math.pi) / 2.0

_CACHE = {}
LAST_RESULTS = None


def _build_nc(reps=1, GQ=GQ):
    import concourse.bacc as bacc
    import concourse.mybir as mybir
    from concourse import tile
    from concourse.tile_rust import add_dep_helper

    fp32 = mybir.dt.float32
    fp16 = mybir.dt.float16
    AF = mybir.ActivationFunctionType
    ALU = mybir.AluOpType

    nc = bacc.Bacc(
        "TRN2", target_bir_lowering=False, debug=False, num_devices=NCORES
    )
    c_d = nc.dram_tensor("c", [KC, P, OS, N], fp32, kind="ExternalInput")
    r_d = nc.dram_tensor("r", [KC, P, OS, N], fp16, kind="ExternalInput")
    a_d = nc.dram_tensor("a", [KC, P, OS, N], fp16, kind="ExternalInput")
    w_d = nc.dram_tensor("w", [KC, P, OS, N], fp16, kind="ExternalInput")
    x_d = nc.dram_tensor("x", [P, KC * B], fp32, kind="ExternalInput")
    oh_d = nc.dram_tensor("oh", [P, B, B], fp16, kind="ExternalInput")
    out_d = nc.dram_tensor("out", [B, OS], fp32, kind="ExternalOutput")

    with tile.TileContext(nc) as tc:
        with (
            tc.tile_pool(name="const", bufs=1) as cpool,
            tc.tile_pool(name="dp", bufs=GQ + 2) as dpool,
            tc.tile_pool(name="qp", bufs=GQ + 2) as qpool,
            tc.tile_pool(name="tp", bufs=GQ + 2) as tpool,
            tc.tile_pool(name="zp", bufs=2) as zpool,
            tc.tile_pool(name="ep", bufs=2) as epool,
            tc.tile_pool(name="psum", bufs=1, space="PSUM") as psp,
            tc.tile_pool(name="outp", bufs=1) as opool,
        ):
            # small tensors first so compute can start early; then const
            # chunks ordered by first use (k=0 before k=1, ...). xn (= -x,
            # the ACT Identity bias) is derived on-chip to avoid another DMA.
            x_sb = cpool.tile([P, KC * B], fp32, tag="x")
            xn_sb = cpool.tile([P, KC * B], fp32, tag="xn")
            oh_sb = cpool.tile([P, B, B], fp16, tag="oh")
            nc.sync.dma_start(x_sb[:], x_d[:])
            nc.sync.dma_start(oh_sb[:], oh_d[:])
            nc.vector.tensor_scalar(
                xn_sb[:], x_sb[:], -1.0, None, op0=mybir.AluOpType.mult
            )
            c_sb, r_sb, a_sb, w_sb = [], [], [], []
            for k in range(KC):
                for lst, dram, nm, dt_ in (
                    (c_sb, c_d, "c", fp32),
                    (r_sb, r_d, "r", fp16),
                    (a_sb, a_d, "a", fp16),
                    (w_sb, w_d, "w", fp16),
                ):
                    t = cpool.tile([P, OS, N], dt_, tag=f"{nm}{k}")
                    nc.sync.dma_start(t[:], dram[k])
                    lst.append(t)

            psum_t = psp.tile([B, OS, N], fp32)
            # pairs: 2 consecutive b, same k
            pairs = [(k, b) for k in range(KC) for b in range(0, B, 2)]
            n_pairs = len(pairs)
            out_sb = opool.tile([B, OS], fp32)

            def bcast2(t):
                return t[:, None].broadcast_to((P, 2, OS, N))

            for rep in range(reps):
                n_mm = 0
                n_d = 0
                total_mm = 8 * n_pairs
                prev_erf = None
                # taper the final groups so the wind-down pipeline is short
                sizes = []
                left = n_pairs
                while left > GQ + 4:
                    sizes.append(GQ)
                    left -= GQ
                while left > 0:
                    s = max(2, min(left, (left + 1) // 2))
                    sizes.append(s)
                    left -= s
                bounds = []
                g0 = 0
                for s in sizes:
                    bounds.append((g0, g0 + s))
                    g0 += s
                for g0, g1 in bounds:
                    grp = pairs[g0:g1]
                    # phase 1: D = c - x (fp32 internal, fp16 out), split
                    # between DVE tensor_scalar (2x_2p) and ACT Identity
                    # (per-partition bias = -x; table-agnostic)
                    ds = []
                    for k, b in grp:
                        d = dpool.tile([P, 2, OS, N], fp16, tag="dp")
                        for j in range(2):
                            col = slice(k * B + b + j, k * B + b + j + 1)
                            if n_d % ACT_D_MOD < ACT_D_LIM:
                                nc.scalar.activation(
                                    d[:, j], c_sb[k][:], AF.Identity,
                                    bias=xn_sb[:, col], scale=1.0,
                                )
                            else:
                                nc.vector.tensor_scalar(
                                    d[:, j], c_sb[k][:], x_sb[:, col], None,
                                    op0=ALU.subtract,
                                )
                            n_d += 1
                        ds.append(d)
                    # phase 2: zm = D*rinv (DVE); e = D_ERF(zm) (ACT table A);
                    #          q = e*wt (DVE)
                    qs = []
                    derfs = []
                    for (k, b), d in zip(grp, ds):
                        zm = zpool.tile([P, 2, OS, N], fp16)
                        nc.vector.tensor_mul(zm[:], d[:], bcast2(r_sb[k]))
                        e = epool.tile([P, 2, OS, N], fp16)
                        ei = nc.scalar.activation(e[:], zm[:], AF.Derivative_Erf)
                        # keep ACT table phases clean: no D_ERF may be
                        # scheduled before the previous group's Erf run ends
                        if prev_erf is not None:
                            add_dep_helper(ei.ins, prev_erf.ins, sync=False,
                                           reason="act table phase order")
                        derfs.append(ei)
                        q = qpool.tile([P, 2, OS, N], fp16, tag="qp")
                        nc.vector.tensor_mul(q[:], e[:], bcast2(w_sb[k]))
                        qs.append(q)
                        # q-stream matmuls issue here (phase 2) to spread PE
                        # load and shrink the end-of-kernel drain
                        for j in range(2):
                            for h in range(2):
                                nc.tensor.matmul(
                                    psum_t[:, 32 * h : 32 * (h + 1), :],
                                    oh_sb[:, b + j, :],
                                    q[:, j, 32 * h : 32 * (h + 1), :],
                                    start=(n_mm < 2),
                                    stop=(n_mm >= total_mm - 2),
                                )
                                n_mm += 1
                    # phase 3: um = D*A4 (DVE); t = Erf(-um) in place (ACT,
                    # table B)
                    ts_ = []
                    for (k, b), d in zip(grp, ds):
                        um = tpool.tile([P, 2, OS, N], fp16, tag="tp")
                        nc.vector.tensor_mul(um[:], d[:], bcast2(a_sb[k]))
                        ti = nc.scalar.activation(um[:], um[:], AF.Erf,
                                                  scale=-1.0)
                        # no Erf before this group's D_ERF run ends
                        add_dep_helper(ti.ins, derfs[-1].ins, sync=False,
                                       reason="act table phase order")
                        prev_erf = ti
                        ts_.append(um)
                    # phase 4: r = q*t overwrites q (DVE, after q's matmuls);
                    # accumulate r stream (PE)
                    for (k, b), q, t_ in zip(grp, qs, ts_):
                        nc.vector.tensor_mul(q[:], q[:], t_[:])
                        for j in range(2):
                            for h in range(2):
                                nc.tensor.matmul(
                                    psum_t[:, 32 * h : 32 * (h + 1), :],
                                    oh_sb[:, b + j, :],
                                    q[:, j, 32 * h : 32 * (h + 1), :],
                                    start=(n_mm < 2),
                                    stop=(n_mm >= total_mm - 2),
                                )
                                n_mm += 1

                nc.vector.tensor_reduce(
                    out_sb[:], psum_t[:], axis=mybir.AxisListType.X, op=ALU.add
                )
            nc.sync.dma_start(out_d[:], out_sb[:])

    nc.compile()
    return nc


def _prep_inputs(x, mx_train, scale, sigma, alpha, w, mx_start):
    c = (np.abs(scale)[:, :, None] * mx_start[None, None, :]
         + mx_train[:, :, None]).astype(np.float32)
    rinv = (1.0 / (np.abs(sigma) + 1e-8)).astype(np.float32)
    r16 = rinv.astype(np.float16)
    A4 = (alpha * rinv * INV_SQRT2).astype(np.float16)
    wt = (w * SQRT_PI_2).astype(np.float16)
    # x packed as [P, KC*B]: xp[p, k*B+b] = x[b, k*128+p]
    xp = np.ascontiguousarray(
        x.T.reshape(KC, P, B).transpose(1, 0, 2).reshape(P, KC * B)
    ).astype(np.float32)
    oh = np.broadcast_to(np.eye(B, dtype=np.float16), (P, B, B))
    oh = np.ascontiguousarray(oh)

    in_maps = []
    for d in range(NCORES):
        sl = slice(d * OS, (d + 1) * OS)
        in_maps.append({
            "c": np.ascontiguousarray(c[:, sl].reshape(KC, P, OS, N)),
            "r": np.ascontiguousarray(r16[:, sl].reshape(KC, P, OS, N)),
            "a": np.ascontiguousarray(A4[:, sl].reshape(KC, P, OS, N)),
            "w": np.ascontiguousarray(wt[:, sl].reshape(KC, P, OS, N)),
            "x": xp,
            "oh": oh,
        })
    return in_maps


def kernel(x, mx_train, scale, sigma, alpha, w, mx_start, _trace=False):
    global LAST_RESULTS
    from concourse.bass_utils import run_bass_kernel_spmd

    if "nc" not in _CACHE:
        _CACHE["nc"] = _build_nc()
    nc = _CACHE["nc"]
    in_maps = _prep_inputs(
        np.asarray(x, np.float32), np.asarray(mx_train, np.float32),
        np.asarray(scale, np.float32), np.asarray(sigma, np.float32),
        np.asarray(alpha, np.float32), np.asarray(w, np.float32),
        np.asarray(mx_start, np.float32),
    )
    res = run_bass_kernel_spmd(nc, in_maps, core_ids=list(range(NCORES)),
                               trace=_trace)
    LAST_RESULTS = res
    return np.concatenate([r["out"] for r in res.results], axis=1)


# revision 21
# speedup vs baseline: 1.6265x; 1.0154x over previous
"""Trainium2 Bass kernel for nn_KATLayer (KAT basis-function layer).

out[b,o] = sum_{i,n} exp(-z^2) * (1 + erf(alpha*z/sqrt(2))) * w[i,o,n]
  z = (x[b,i] - c[i,o,n]) / (|sigma|+1e-8),  c = |scale|*mx_start + mx_train

Sharding: output dim O split across 8 cores (O_shard=64). Per core:
  partitions = i (4 chunks of 128), free = (o_local, n) = 1024 per tile,
  tiles processed in QUADS (4 consecutive b, same i-chunk) so elementwise
  and activation ops run at free=4096, amortizing fixed overheads.

Math (all intermediates fp16; validated ~5e-4 rel err vs the 2e-2 gate):
  zm = (c - x)*rinv     [= -z; the c - x subtraction is the only
                         cancellation-sensitive step (fp32 internal).
                         Two alternatives, mixed ~half/half to balance
                         engines: (b)-quads let ACT compute d = c - x via
                         Identity with per-partition bias -x (Identity is
                         in EVERY act table set -> no table switch), then
                         DVE zm = d*rinv16 (fp16 TT 2x). (c)-quads use a
                         DVE scalar_tensor_tensor (1x) directly.]
  DVE:  um = zm*A3      [A3 = alpha/sqrt(2), fp16 TT 2x; = -alpha*z/sqrt2]
  ACT:  e  = Derivative_Erf(zm)    [= 2/sqrt(pi)*exp(-z^2), even in z]
  DVE:  q  = e*wt       [in place on e; wt = w*sqrt(pi)/2]
  ACT:  t  = Erf(-um)   [in place on um; = erf(alpha*z/sqrt(2))]
  DVE:  r  = q*t        [in place on q]
  PE :  psum += onehot_b.T @ q (right after q) ; psum += onehot_b.T @ r
        (the "+1" of (1+erf) is absorbed by accumulating BOTH q and r
        streams in PSUM — no fp16 STT, which only has a 1x uop)
Final: DVE reduce over n: psum(32,64,16) -> (32,64); DMA out.

Per-k consts (rinv16, A3, wt) are read through stride-0 broadcast APs
across the quad dim. Derivative_Erf and Erf live in different ACT table
sets (~2.7us/switch), so quads are processed in groups with phase-batched
activations (2 switches per group), enforced with no-sync scheduler edges.
"""
import sys

sys.path.insert(0, "/opt/trn_rl_repo")
import math

import numpy as np

B, I, O, N = 32, 512, 512, 16
NCORES = 8
OS = O // NCORES          # 64 output dims per core
KC = I // 128             # 4 i-chunks
P = 128
Q = 4                     # b's per quad
GQ = 4                    # quads per activation-phase group (16 tiles)
INV_SQRT2 = 0.7071067811865476
SQRT_PI_2 = math.sqrt(math.pi) / 2.0

_CACHE = {}
LAST_RESULTS = None


def _build_nc(reps=1, GQ=GQ):
    import concourse.bacc as bacc
    import concourse.mybir as mybir
    from concourse import tile
    from concourse.tile_rust import add_dep_helper

    fp32 = mybir.dt.float32
    fp16 = mybir.dt.float16
    AF = mybir.ActivationFunctionType
    ALU = mybir.AluOpType

    nc = bacc.Bacc(
        "TRN2", target_bir_lowering=False, debug=False, num_devices=NCORES
    )
    c_d = nc.dram_tensor("c", [KC, P, OS, N], fp32, kind="ExternalInput")
    r_d = nc.dram_tensor("r", [KC, P, OS, N], fp16, kind="ExternalInput")
    a_d = nc.dram_tensor("a", [KC, P, OS, N], fp16, kind="ExternalInput")
    w_d = nc.dram_tensor("w", [KC, P, OS, N], fp16, kind="ExternalInput")
    x_d = nc.dram_tensor("x", [P, KC * B], fp32, kind="ExternalInput")
    oh_d = nc.dram_tensor("oh", [P, B, B], fp16, kind="ExternalInput")
    out_d = nc.dram_tensor("out", [B, OS], fp32, kind="ExternalOutput")

    with tile.TileContext(nc) as tc:
        with (
            tc.tile_pool(name="const", bufs=1) as cpool,
            tc.tile_pool(name="dp", bufs=3) as dpool,
            tc.tile_pool(name="zp", bufs=3) as zpool,
            tc.tile_pool(name="eq", bufs=GQ + 1) as eqpool,
            tc.tile_pool(name="tp", bufs=GQ + 2) as tpool,
            tc.tile_pool(name="psum", bufs=1, space="PSUM") as psp,
            tc.tile_pool(name="outp", bufs=1) as opool,
        ):
            # small tensors first so compute can start early; then const
            # chunks ordered by first use (k=0 before k=1, ...). xn (= -x,
            # the ACT Identity bias) is derived on-chip to avoid more DMAs.
            x_sb = cpool.tile([P, KC * B], fp32, tag="x")
            xn_sb = cpool.tile([P, KC * B], fp32, tag="xn")
            oh_sb = cpool.tile([P, B, B], fp16, tag="oh")
            nc.sync.dma_start(x_sb[:], x_d[:])
            nc.sync.dma_start(oh_sb[:], oh_d[:])
            nc.vector.tensor_scalar(
                xn_sb[:], x_sb[:], -1.0, None, op0=mybir.AluOpType.mult
            )
            c_sb, r_sb, a_sb, w_sb = [], [], [], []
            for k in range(KC):
                for lst, dram, nm, dt_ in (
                    (c_sb, c_d, "c", fp32),
                    (r_sb, r_d, "r", fp16),
                    (a_sb, a_d, "a", fp16),
                    (w_sb, w_d, "w", fp16),
                ):
                    t = cpool.tile([P, OS, N], dt_, tag=f"{nm}{k}")
                    nc.sync.dma_start(t[:], dram[k])
                    lst.append(t)

            psum_t = psp.tile([B, OS, N], fp32)
            # quads: 4 consecutive b, same k
            quads = [(k, b) for k in range(KC) for b in range(0, B, Q)]
            n_quads = len(quads)
            # ~15/32 of quads compute d = c - x on ACT (engine balance)
            act_d = [(qi % 2 == 0) and qi != 30 for qi in range(n_quads)]
            out_sb = opool.tile([B, OS], fp32)

            def bcastq(t):
                return t[:, None].broadcast_to((P, Q, OS, N))

            for rep in range(reps):
                n_mm = 0
                total_mm = 2 * Q * 2 * n_quads
                prev_erf = None
                sizes = [GQ] * 7 + [2, 1, 1]
                assert sum(sizes) == n_quads
                bounds = []
                g0 = 0
                for s in sizes:
                    bounds.append((g0, g0 + s))
                    g0 += s
                for g0, g1 in bounds:
                    grp = list(range(g0, g1))
                    # phase 1: (b)-quads: d = c - x via ACT Identity
                    ds = {}
                    for qi in grp:
                        k, b = quads[qi]
                        if not act_d[qi]:
                            continue
                        d = dpool.tile([P, Q, OS, N], fp16, tag="dp")
                        for j in range(Q):
                            col = slice(k * B + b + j, k * B + b + j + 1)
                            nc.scalar.activation(
                                d[:, j], c_sb[k][:], AF.Identity,
                                bias=xn_sb[:, col], scale=1.0,
                            )
                        ds[qi] = d
                    # phase 2: zm quads; um = zm*A3 (DVE)
                    ums, zms = {}, {}
                    for qi in grp:
                        k, b = quads[qi]
                        zq = zpool.tile([P, Q, OS, N], fp16, tag="zp")
                        if act_d[qi]:
                            nc.vector.tensor_mul(
                                zq[:], ds[qi][:], bcastq(r_sb[k])
                            )
                        else:
                            for j in range(Q):
                                col = slice(k * B + b + j, k * B + b + j + 1)
                                nc.vector.scalar_tensor_tensor(
                                    zq[:, j], c_sb[k][:], x_sb[:, col],
                                    r_sb[k][:],
                                    op0=ALU.subtract, op1=ALU.mult,
                                )
                        um = tpool.tile([P, Q, OS, N], fp16, tag="tp")
                        nc.vector.tensor_mul(um[:], zq[:], bcastq(a_sb[k]))
                        zms[qi], ums[qi] = zq, um
                    # phase 3: e = D_ERF(zm) (ACT table A); q = e*wt in
                    # place (DVE); q-stream matmuls (PE)
                    qs = {}
                    derfs = []
                    for qi in grp:
                        k, b = quads[qi]
                        e = eqpool.tile([P, Q, OS, N], fp16, tag="eq")
                        ei = nc.scalar.activation(
                            e[:], zms[qi][:], AF.Derivative_Erf
                        )
                        if prev_erf is not None:
                            add_dep_helper(ei.ins, prev_erf.ins, sync=False,
                                           reason="act table phase order")
                        derfs.append(ei)
                        nc.vector.tensor_mul(e[:], e[:], bcastq(w_sb[k]))
                        qs[qi] = e
                        for j in range(Q):
                            for h in range(2):
                                nc.tensor.matmul(
                                    psum_t[:, 32 * h : 32 * (h + 1), :],
                                    oh_sb[:, b + j, :],
                                    e[:, j, 32 * h : 32 * (h + 1), :],
                                    start=(n_mm < 2),
                                    stop=(n_mm >= total_mm - 2),
                                )
                                n_mm += 1
                    # phase 4: t = Erf(-um) in place (ACT table B)
                    for qi in grp:
                        um = ums[qi]
                        ti = nc.scalar.activation(um[:], um[:], AF.Erf,
                                                  scale=-1.0)
                        add_dep_helper(ti.ins, derfs[-1].ins, sync=False,
                                       reason="act table phase order")
                        prev_erf = ti
                    # phase 5: r = q*t in place on q (DVE, after q's
                    # matmuls); r-stream matmuls (PE)
                    for qi in grp:
                        k, b = quads[qi]
                        q_, t_ = qs[qi], ums[qi]
                        nc.vector.tensor_mul(q_[:], q_[:], t_[:])
                        for j in range(Q):
                            for h in range(2):
                                nc.tensor.matmul(
                                    psum_t[:, 32 * h : 32 * (h + 1), :],
                                    oh_sb[:, b + j, :],
                                    q_[:, j, 32 * h : 32 * (h + 1), :],
                                    start=(n_mm < 2),
                                    stop=(n_mm >= total_mm - 2),
                                )
                                n_mm += 1

                nc.vector.tensor_reduce(
                    out_sb[:], psum_t[:], axis=mybir.AxisListType.X, op=ALU.add
                )
            nc.sync.dma_start(out_d[:], out_sb[:])

    nc.compile()
    return nc


def _prep_inputs(x, mx_train, scale, sigma, alpha, w, mx_start):
    c = (np.abs(scale)[:, :, None] * mx_start[None, None, :]
         + mx_train[:, :, None]).astype(np.float32)
    rinv = (1.0 / (np.abs(sigma) + 1e-8)).astype(np.float32)
    r16 = rinv.astype(np.float16)
    A3 = (alpha * INV_SQRT2).astype(np.float16)
    wt = (w * SQRT_PI_2).astype(np.float16)
    # x packed as [P, KC*B]: xp[p, k*B+b] = x[b, k*128+p]
    xp = np.ascontiguousarray(
        x.T.reshape(KC, P, B).transpose(1, 0, 2).reshape(P, KC * B)
    ).astype(np.float32)
    oh = np.broadcast_to(np.eye(B, dtype=np.float16), (P, B, B))
    oh = np.ascontiguousarray(oh)

    in_maps = []
    for d in range(NCORES):
        sl = slice(d * OS, (d + 1) * OS)
        in_maps.append({
            "c": np.ascontiguousarray(c[:, sl].reshape(KC, P, OS, N)),
            "r": np.ascontiguousarray(r16[:, sl].reshape(KC, P, OS, N)),
            "a": np.ascontiguousarray(A3[:, sl].reshape(KC, P, OS, N)),
            "w": np.ascontiguousarray(wt[:, sl].reshape(KC, P, OS, N)),
            "x": xp,
            "oh": oh,
        })
    return in_maps


def kernel(x, mx_train, scale, sigma, alpha, w, mx_start, _trace=False):
    global LAST_RESULTS
    from concourse.bass_utils import run_bass_kernel_spmd

    if "nc" not in _CACHE:
        _CACHE["nc"] = _build_nc()
    nc = _CACHE["nc"]
    in_maps = _prep_inputs(
        np.asarray(x, np.float32), np.asarray(mx_train, np.float32),
        np.asarray(scale, np.float32), np.asarray(sigma, np.float32),
        np.asarray(alpha, np.float32), np.asarray(w, np.float32),
        np.asarray(mx_start, np.float32),
    )
    res = run_bass_kernel_spmd(nc, in_maps, core_ids=list(range(NCORES)),
                               trace=_trace)
    LAST_RESULTS = res
    return np.concatenate([r["out"] for r in res.results], axis=1)


# revision 23
# speedup vs baseline: 1.6387x; 1.0075x over previous
"""Trainium2 Bass kernel for nn_KATLayer (KAT basis-function layer).

out[b,o] = sum_{i,n} exp(-z^2) * (1 + erf(alpha*z/sqrt(2))) * w[i,o,n]
  z = (x[b,i] - c[i,o,n]) / (|sigma|+1e-8),  c = |scale|*mx_start + mx_train

Sharding: output dim O split across 8 cores (O_shard=64). Per core:
  partitions = i (4 chunks of 128), free = (o_local, n) = 1024 per tile,
  tiles processed in QUADS (4 consecutive b, same i-chunk) so elementwise
  and activation ops run at free=4096, amortizing fixed overheads.

Math (all intermediates fp16; validated ~5e-4 rel err vs the 2e-2 gate):
  zm = (c - x)*rinv     [= -z; the c - x subtraction is the only
                         cancellation-sensitive step (fp32 internal).
                         Two alternatives, mixed ~half/half to balance
                         engines: (b)-quads let ACT compute d = c - x via
                         Identity with per-partition bias -x (Identity is
                         in EVERY act table set -> no table switch), then
                         DVE zm = d*rinv16 (fp16 TT 2x). (c)-quads use a
                         DVE scalar_tensor_tensor (1x) directly.]
  DVE:  um = zm*A3      [A3 = alpha/sqrt(2), fp16 TT 2x; = -alpha*z/sqrt2]
  ACT:  e  = Derivative_Erf(zm)    [= 2/sqrt(pi)*exp(-z^2), even in z]
  DVE:  q  = e*wt       [in place on e; wt = w*sqrt(pi)/2]
  ACT:  t  = Erf(-um)   [in place on um; = erf(alpha*z/sqrt(2))]
  DVE:  r  = q*t        [in place on q]
  PE :  psum += onehot_b.T @ q (right after q) ; psum += onehot_b.T @ r
        (the "+1" of (1+erf) is absorbed by accumulating BOTH q and r
        streams in PSUM — no fp16 STT, which only has a 1x uop)
Final: DVE reduce over n: psum(32,64,16) -> (32,64); DMA out.

Per-k consts (rinv16, A3, wt) are read through stride-0 broadcast APs
across the quad dim. Derivative_Erf and Erf live in different ACT table
sets (~2.7us/switch), so quads are processed in groups with phase-batched
activations (2 switches per group), enforced with no-sync scheduler edges.
"""
import sys

sys.path.insert(0, "/opt/trn_rl_repo")
import math

import numpy as np

B, I, O, N = 32, 512, 512, 16
NCORES = 8
OS = O // NCORES          # 64 output dims per core
KC = I // 128             # 4 i-chunks
P = 128
Q = 4                     # b's per quad
GQ = 4                    # quads per activation-phase group (16 tiles)
INV_SQRT2 = 0.7071067811865476
SQRT_PI_2 = math.sqrt(math.pi) / 2.0

_CACHE = {}
LAST_RESULTS = None


def _build_nc(reps=1, GQ=GQ):
    import concourse.bacc as bacc
    import concourse.mybir as mybir
    from concourse import tile
    from concourse.tile_rust import add_dep_helper

    fp32 = mybir.dt.float32
    fp16 = mybir.dt.float16
    AF = mybir.ActivationFunctionType
    ALU = mybir.AluOpType

    nc = bacc.Bacc(
        "TRN2", target_bir_lowering=False, debug=False, num_devices=NCORES
    )
    c_d = nc.dram_tensor("c", [KC, P, OS, N], fp32, kind="ExternalInput")
    r_d = nc.dram_tensor("r", [KC, P, OS, N], fp16, kind="ExternalInput")
    a_d = nc.dram_tensor("a", [KC, P, OS, N], fp16, kind="ExternalInput")
    w_d = nc.dram_tensor("w", [KC, P, OS, N], fp16, kind="ExternalInput")
    x_d = nc.dram_tensor("x", [P, KC * B], fp32, kind="ExternalInput")
    oh_d = nc.dram_tensor("oh", [P, B, B], fp16, kind="ExternalInput")
    out_d = nc.dram_tensor("out", [B, OS], fp32, kind="ExternalOutput")

    with tile.TileContext(nc) as tc:
        with (
            tc.tile_pool(name="const", bufs=1) as cpool,
            tc.tile_pool(name="dp", bufs=3) as dpool,
            tc.tile_pool(name="zp", bufs=3) as zpool,
            tc.tile_pool(name="eq", bufs=GQ + 1) as eqpool,
            tc.tile_pool(name="tp", bufs=GQ + 2) as tpool,
            tc.tile_pool(name="psum", bufs=1, space="PSUM") as psp,
            tc.tile_pool(name="outp", bufs=1) as opool,
        ):
            # small tensors first so compute can start early; then const
            # chunks ordered by first use (k=0 before k=1, ...). xn (= -x,
            # the ACT Identity bias) is derived on-chip to avoid more DMAs.
            x_sb = cpool.tile([P, KC * B], fp32, tag="x")
            xn_sb = cpool.tile([P, KC * B], fp32, tag="xn")
            oh_sb = cpool.tile([P, B, B], fp16, tag="oh")
            nc.sync.dma_start(x_sb[:], x_d[:])
            nc.sync.dma_start(oh_sb[:], oh_d[:])
            nc.vector.tensor_scalar(
                xn_sb[:], x_sb[:], -1.0, None, op0=mybir.AluOpType.mult
            )
            c_sb, r_sb, a_sb, w_sb = [], [], [], []
            for k in range(KC):
                for lst, dram, nm, dt_ in (
                    (c_sb, c_d, "c", fp32),
                    (r_sb, r_d, "r", fp16),
                    (a_sb, a_d, "a", fp16),
                    (w_sb, w_d, "w", fp16),
                ):
                    t = cpool.tile([P, OS, N], dt_, tag=f"{nm}{k}")
                    nc.sync.dma_start(t[:], dram[k])
                    lst.append(t)

            psum_t = psp.tile([B, OS, N], fp32)
            # quads: 4 consecutive b, same k
            quads = [(k, b) for k in range(KC) for b in range(0, B, Q)]
            n_quads = len(quads)
            # ~15/32 of quads compute d = c - x on ACT (engine balance);
            # first and last quads stay on DVE so the startup ramp and the
            # tail drain don't wait on the ACT Identity chain
            act_d = [(qi % 2 == 1 and qi < 28) or qi == 2
                     for qi in range(n_quads)]
            out_sb = opool.tile([B, OS], fp32)

            def bcastq(t):
                return t[:, None].broadcast_to((P, Q, OS, N))

            for rep in range(reps):
                n_mm = 0
                total_mm = 2 * Q * 2 * n_quads
                prev_erf = None
                sizes = [GQ] * 7 + [2, 2]
                assert sum(sizes) == n_quads
                bounds = []
                g0 = 0
                for s in sizes:
                    bounds.append((g0, g0 + s))
                    g0 += s
                for g0, g1 in bounds:
                    grp = list(range(g0, g1))
                    # phase 1: (b)-quads: d = c - x via ACT Identity
                    ds = {}
                    for qi in grp:
                        k, b = quads[qi]
                        if not act_d[qi]:
                            continue
                        d = dpool.tile([P, Q, OS, N], fp16, tag="dp")
                        for j in range(Q):
                            col = slice(k * B + b + j, k * B + b + j + 1)
                            nc.scalar.activation(
                                d[:, j], c_sb[k][:], AF.Identity,
                                bias=xn_sb[:, col], scale=1.0,
                            )
                        ds[qi] = d
                    # phase 2: zm quads; um = zm*A3 (DVE)
                    ums, zms = {}, {}
                    for qi in grp:
                        k, b = quads[qi]
                        zq = zpool.tile([P, Q, OS, N], fp16, tag="zp")
                        if act_d[qi]:
                            nc.vector.tensor_mul(
                                zq[:], ds[qi][:], bcastq(r_sb[k])
                            )
                        else:
                            for j in range(Q):
                                col = slice(k * B + b + j, k * B + b + j + 1)
                                nc.vector.scalar_tensor_tensor(
                                    zq[:, j], c_sb[k][:], x_sb[:, col],
                                    r_sb[k][:],
                                    op0=ALU.subtract, op1=ALU.mult,
                                )
                        um = tpool.tile([P, Q, OS, N], fp16, tag="tp")
                        nc.vector.tensor_mul(um[:], zq[:], bcastq(a_sb[k]))
                        zms[qi], ums[qi] = zq, um
                    # phase 3: e = D_ERF(zm) (ACT table A); q = e*wt in
                    # place (DVE); q-stream matmuls (PE)
                    qs = {}
                    derfs = []
                    for qi in grp:
                        k, b = quads[qi]
                        e = eqpool.tile([P, Q, OS, N], fp16, tag="eq")
                        ei = nc.scalar.activation(
                            e[:], zms[qi][:], AF.Derivative_Erf
                        )
                        if prev_erf is not None:
                            add_dep_helper(ei.ins, prev_erf.ins, sync=False,
                                           reason="act table phase order")
                        derfs.append(ei)
                        nc.vector.tensor_mul(e[:], e[:], bcastq(w_sb[k]))
                        qs[qi] = e
                        for j in range(Q):
                            for h in range(2):
                                nc.tensor.matmul(
                                    psum_t[:, 32 * h : 32 * (h + 1), :],
                                    oh_sb[:, b + j, :],
                                    e[:, j, 32 * h : 32 * (h + 1), :],
                                    start=(n_mm < 2),
                                    stop=(n_mm >= total_mm - 2),
                                )
                                n_mm += 1
                    # phase 4: t = Erf(-um) in place (ACT table B)
                    for qi in grp:
                        um = ums[qi]
                        ti = nc.scalar.activation(um[:], um[:], AF.Erf,
                                                  scale=-1.0)
                        add_dep_helper(ti.ins, derfs[-1].ins, sync=False,
                                       reason="act table phase order")
                        prev_erf = ti
                    # phase 5: r = q*t in place on q (DVE, after q's
                    # matmuls); r-stream matmuls (PE)
                    for qi in grp:
                        k, b = quads[qi]
                        q_, t_ = qs[qi], ums[qi]
                        nc.vector.tensor_mul(q_[:], q_[:], t_[:])
                        for j in range(Q):
                            for h in range(2):
                                nc.tensor.matmul(
                                    psum_t[:, 32 * h : 32 * (h + 1), :],
                                    oh_sb[:, b + j, :],
                                    q_[:, j, 32 * h : 32 * (h + 1), :],
                                    start=(n_mm < 2),
                                    stop=(n_mm >= total_mm - 2),
                                )
                                n_mm += 1

                nc.vector.tensor_reduce(
                    out_sb[:], psum_t[:], axis=mybir.AxisListType.X, op=ALU.add
                )
            nc.sync.dma_start(out_d[:], out_sb[:])

    nc.compile()
    return nc


def _prep_inputs(x, mx_train, scale, sigma, alpha, w, mx_start):
    c = (np.abs(scale)[:, :, None] * mx_start[None, None, :]
         + mx_train[:, :, None]).astype(np.float32)
    rinv = (1.0 / (np.abs(sigma) + 1e-8)).astype(np.float32)
    r16 = rinv.astype(np.float16)
    A3 = (alpha * INV_SQRT2).astype(np.float16)
    wt = (w * SQRT_PI_2).astype(np.float16)
    # x packed as [P, KC*B]: xp[p, k*B+b] = x[b, k*128+p]
    xp = np.ascontiguousarray(
        x.T.reshape(KC, P, B).transpose(1, 0, 2).reshape(P, KC * B)
    ).astype(np.float32)
    oh = np.broadcast_to(np.eye(B, dtype=np.float16), (P, B, B))
    oh = np.ascontiguousarray(oh)

    in_maps = []
    for d in range(NCORES):
        sl = slice(d * OS, (d + 1) * OS)
        in_maps.append({
            "c": np.ascontiguousarray(c[:, sl].reshape(KC, P, OS, N)),
            "r": np.ascontiguousarray(r16[:, sl].reshape(KC, P, OS, N)),
            "a": np.ascontiguousarray(A3[:, sl].reshape(KC, P, OS, N)),
            "w": np.ascontiguousarray(wt[:, sl].reshape(KC, P, OS, N)),
            "x": xp,
            "oh": oh,
        })
    return in_maps


def kernel(x, mx_train, scale, sigma, alpha, w, mx_start, _trace=False):
    global LAST_RESULTS
    from concourse.bass_utils import run_bass_kernel_spmd

    if "nc" not in _CACHE:
        _CACHE["nc"] = _build_nc()
    nc = _CACHE["nc"]
    in_maps = _prep_inputs(
        np.asarray(x, np.float32), np.asarray(mx_train, np.float32),
        np.asarray(scale, np.float32), np.asarray(sigma, np.float32),
        np.asarray(alpha, np.float32), np.asarray(w, np.float32),
        np.asarray(mx_start, np.float32),
    )
    res = run_bass_kernel_spmd(nc, in_maps, core_ids=list(range(NCORES)),
                               trace=_trace)
    LAST_RESULTS = res
    return np.concatenate([r["out"] for r in res.results], axis=1)


# revision 27
# speedup vs baseline: 1.6473x; 1.0053x over previous
"""Trainium2 Bass kernel for nn_KATLayer (KAT basis-function layer).

out[b,o] = sum_{i,n} exp(-z^2) * (1 + erf(alpha*z/sqrt(2))) * w[i,o,n]
  z = (x[b,i] - c[i,o,n]) / (|sigma|+1e-8),  c = |scale|*mx_start + mx_train

Sharding: output dim O split across 8 cores (O_shard=64). Per core:
  partitions = i (4 chunks of 128), free = (o_local, n) = 1024 per tile,
  tiles processed in QUADS (4 consecutive b, same i-chunk) so elementwise
  and activation ops run at free=4096, amortizing fixed overheads.

Math (all intermediates fp16; validated ~5e-4 rel err vs the 2e-2 gate):
  zm = (c - x)*rinv     [= -z; the c - x subtraction is the only
                         cancellation-sensitive step (fp32 internal).
                         Two alternatives, mixed ~half/half to balance
                         engines: (b)-quads let ACT compute d = c - x via
                         Identity with per-partition bias -x (Identity is
                         in EVERY act table set -> no table switch), then
                         DVE zm = d*rinv16 (fp16 TT 2x). (c)-quads use a
                         DVE scalar_tensor_tensor (1x) directly.]
  DVE:  um = zm*A3      [A3 = alpha/sqrt(2), fp16 TT 2x; = -alpha*z/sqrt2]
  ACT:  e  = Derivative_Erf(zm)    [= 2/sqrt(pi)*exp(-z^2), even in z]
  DVE:  q  = e*wt       [in place on e; wt = w*sqrt(pi)/2]
  ACT:  t  = Erf(-um)   [in place on um; = erf(alpha*z/sqrt(2))]
  DVE:  r  = q*t        [in place on q]
  PE :  psum += onehot_b.T @ q (right after q) ; psum += onehot_b.T @ r
        (the "+1" of (1+erf) is absorbed by accumulating BOTH q and r
        streams in PSUM — no fp16 STT, which only has a 1x uop)
Final: DVE reduce over n: psum(32,64,16) -> (32,64); DMA out.

Per-k consts (rinv16, A3, wt) are read through stride-0 broadcast APs
across the quad dim. Derivative_Erf and Erf live in different ACT table
sets (~2.7us/switch), so quads are processed in groups with phase-batched
activations (2 switches per group), enforced with no-sync scheduler edges.
"""
import sys

sys.path.insert(0, "/opt/trn_rl_repo")
import math

import numpy as np

B, I, O, N = 32, 512, 512, 16
NCORES = 8
OS = O // NCORES          # 64 output dims per core
KC = I // 128             # 4 i-chunks
P = 128
Q = 4                     # b's per quad
GQ = 4                    # quads per activation-phase group (16 tiles)
INV_SQRT2 = 0.7071067811865476
SQRT_PI_2 = math.sqrt(math.pi) / 2.0

_CACHE = {}
LAST_RESULTS = None


def _build_nc(reps=1, GQ=GQ):
    import concourse.bacc as bacc
    import concourse.mybir as mybir
    from concourse import tile
    from concourse.tile_rust import add_dep_helper

    fp32 = mybir.dt.float32
    fp16 = mybir.dt.float16
    AF = mybir.ActivationFunctionType
    ALU = mybir.AluOpType

    nc = bacc.Bacc(
        "TRN2", target_bir_lowering=False, debug=False, num_devices=NCORES
    )
    c_d = nc.dram_tensor("c", [KC, P, OS, N], fp32, kind="ExternalInput")
    r_d = nc.dram_tensor("r", [KC, P, OS, N], fp16, kind="ExternalInput")
    a_d = nc.dram_tensor("a", [KC, P, OS, N], fp16, kind="ExternalInput")
    w_d = nc.dram_tensor("w", [KC, P, OS, N], fp16, kind="ExternalInput")
    x_d = nc.dram_tensor("x", [P, KC * B], fp32, kind="ExternalInput")
    oh_d = nc.dram_tensor("oh", [P, B, B], fp16, kind="ExternalInput")
    out_d = nc.dram_tensor("out", [B, OS], fp32, kind="ExternalOutput")

    with tile.TileContext(nc) as tc:
        with (
            tc.tile_pool(name="const", bufs=1) as cpool,
            tc.tile_pool(name="dp", bufs=3) as dpool,
            tc.tile_pool(name="zp", bufs=3) as zpool,
            tc.tile_pool(name="eq", bufs=GQ + 1) as eqpool,
            tc.tile_pool(name="tp", bufs=GQ + 2) as tpool,
            tc.tile_pool(name="psum", bufs=1, space="PSUM") as psp,
            tc.tile_pool(name="outp", bufs=1) as opool,
        ):
            # small tensors first so compute can start early; then const
            # chunks ordered by first use (k=0 before k=1, ...). xn (= -x,
            # the ACT Identity bias) is derived on-chip to avoid more DMAs.
            x_sb = cpool.tile([P, KC * B], fp32, tag="x")
            xn_sb = cpool.tile([P, KC * B], fp32, tag="xn")
            oh_sb = cpool.tile([P, B, B], fp16, tag="oh")
            nc.sync.dma_start(x_sb[:], x_d[:])
            nc.vector.tensor_scalar(
                xn_sb[:], x_sb[:], -1.0, None, op0=mybir.AluOpType.mult
            )
            c_sb, r_sb, a_sb, w_sb = [], [], [], []
            for k in range(KC):
                for lst, dram, nm, dt_ in (
                    (c_sb, c_d, "c", fp32),
                    (r_sb, r_d, "r", fp16),
                    (a_sb, a_d, "a", fp16),
                    (w_sb, w_d, "w", fp16),
                ):
                    t = cpool.tile([P, OS, N], dt_, tag=f"{nm}{k}")
                    lst.append(t)
                nc.sync.dma_start(c_sb[k][:], c_d[k])
                nc.sync.dma_start(r_sb[k][:], r_d[k])
                if k == 0:
                    nc.sync.dma_start(oh_sb[:], oh_d[:])
                nc.sync.dma_start(a_sb[k][:], a_d[k])
                nc.sync.dma_start(w_sb[k][:], w_d[k])

            psum_t = psp.tile([B, OS, N], fp32)
            # quads: 4 consecutive b, same k
            quads = [(k, b) for k in range(KC) for b in range(0, B, Q)]
            n_quads = len(quads)
            # ~15/32 of quads compute d = c - x on ACT (engine balance);
            # first and last quads stay on DVE so the startup ramp and the
            # tail drain don't wait on the ACT Identity chain
            act_d = [(qi % 2 == 1 and qi < 28) or qi == 28
                     for qi in range(n_quads)]
            out_sb = opool.tile([B, OS], fp32)

            def bcastq(t):
                return t[:, None].broadcast_to((P, Q, OS, N))

            for rep in range(reps):
                n_mm = 0
                total_mm = 2 * Q * 2 * n_quads
                prev_erf = None
                sizes = [GQ] * 7 + [2, 2]
                assert sum(sizes) == n_quads
                bounds = []
                g0 = 0
                for s in sizes:
                    bounds.append((g0, g0 + s))
                    g0 += s
                for g0, g1 in bounds:
                    grp = list(range(g0, g1))
                    # phase 1: (b)-quads: d = c - x via ACT Identity
                    ds = {}
                    for qi in grp:
                        k, b = quads[qi]
                        if not act_d[qi]:
                            continue
                        d = dpool.tile([P, Q, OS, N], fp16, tag="dp")
                        for j in range(Q):
                            col = slice(k * B + b + j, k * B + b + j + 1)
                            nc.scalar.activation(
                                d[:, j], c_sb[k][:], AF.Identity,
                                bias=xn_sb[:, col], scale=1.0,
                            )
                        ds[qi] = d
                    # phase 2: zm quads; um = zm*A3 (DVE)
                    ums, zms = {}, {}
                    for qi in grp:
                        k, b = quads[qi]
                        zq = zpool.tile([P, Q, OS, N], fp16, tag="zp")
                        if act_d[qi]:
                            nc.vector.tensor_mul(
                                zq[:], ds[qi][:], bcastq(r_sb[k])
                            )
                        else:
                            for j in range(Q):
                                col = slice(k * B + b + j, k * B + b + j + 1)
                                nc.vector.scalar_tensor_tensor(
                                    zq[:, j], c_sb[k][:], x_sb[:, col],
                                    r_sb[k][:],
                                    op0=ALU.subtract, op1=ALU.mult,
                                )
                        um = tpool.tile([P, Q, OS, N], fp16, tag="tp")
                        nc.vector.tensor_mul(um[:], zq[:], bcastq(a_sb[k]))
                        zms[qi], ums[qi] = zq, um
                    # phase 3: e = D_ERF(zm) (ACT table A); q = e*wt in
                    # place (DVE); q-stream matmuls (PE)
                    qs = {}
                    derfs = []
                    for qi in grp:
                        k, b = quads[qi]
                        e = eqpool.tile([P, Q, OS, N], fp16, tag="eq")
                        ei = nc.scalar.activation(
                            e[:], zms[qi][:], AF.Derivative_Erf
                        )
                        if prev_erf is not None:
                            add_dep_helper(ei.ins, prev_erf.ins, sync=False,
                                           reason="act table phase order")
                        derfs.append(ei)
                        nc.vector.tensor_mul(e[:], e[:], bcastq(w_sb[k]))
                        qs[qi] = e
                        for j in range(Q):
                            for h in range(2):
                                nc.tensor.matmul(
                                    psum_t[:, 32 * h : 32 * (h + 1), :],
                                    oh_sb[:, b + j, :],
                                    e[:, j, 32 * h : 32 * (h + 1), :],
                                    start=(n_mm < 2),
                                    stop=(n_mm >= total_mm - 2),
                                )
                                n_mm += 1
                    # phase 4: t = Erf(-um) in place (ACT table B)
                    for qi in grp:
                        um = ums[qi]
                        ti = nc.scalar.activation(um[:], um[:], AF.Erf,
                                                  scale=-1.0)
                        add_dep_helper(ti.ins, derfs[-1].ins, sync=False,
                                       reason="act table phase order")
                        prev_erf = ti
                    # phase 5: r = q*t in place on q (DVE, after q's
                    # matmuls); r-stream matmuls (PE)
                    for qi in grp:
                        k, b = quads[qi]
                        q_, t_ = qs[qi], ums[qi]
                        nc.vector.tensor_mul(q_[:], q_[:], t_[:])
                        for j in range(Q):
                            for h in range(2):
                                nc.tensor.matmul(
                                    psum_t[:, 32 * h : 32 * (h + 1), :],
                                    oh_sb[:, b + j, :],
                                    q_[:, j, 32 * h : 32 * (h + 1), :],
                                    start=(n_mm < 2),
                                    stop=(n_mm >= total_mm - 2),
                                )
                                n_mm += 1

                nc.vector.tensor_reduce(
                    out_sb[:], psum_t[:], axis=mybir.AxisListType.X, op=ALU.add
                )
            # issue the out DMA from the ACT queue — it is idle by the time
            # the final reduce lands, unlike the busy sync queue
            nc.scalar.dma_start(out_d[:], out_sb[:])

    nc.compile()
    return nc


def _prep_inputs(x, mx_train, scale, sigma, alpha, w, mx_start):
    c = (np.abs(scale)[:, :, None] * mx_start[None, None, :]
         + mx_train[:, :, None]).astype(np.float32)
    rinv = (1.0 / (np.abs(sigma) + 1e-8)).astype(np.float32)
    r16 = rinv.astype(np.float16)
    A3 = (alpha * INV_SQRT2).astype(np.float16)
    wt = (w * SQRT_PI_2).astype(np.float16)
    # x packed as [P, KC*B]: xp[p, k*B+b] = x[b, k*128+p]
    xp = np.ascontiguousarray(
        x.T.reshape(KC, P, B).transpose(1, 0, 2).reshape(P, KC * B)
    ).astype(np.float32)
    oh = np.broadcast_to(np.eye(B, dtype=np.float16), (P, B, B))
    oh = np.ascontiguousarray(oh)

    in_maps = []
    for d in range(NCORES):
        sl = slice(d * OS, (d + 1) * OS)
        in_maps.append({
            "c": np.ascontiguousarray(c[:, sl].reshape(KC, P, OS, N)),
            "r": np.ascontiguousarray(r16[:, sl].reshape(KC, P, OS, N)),
            "a": np.ascontiguousarray(A3[:, sl].reshape(KC, P, OS, N)),
            "w": np.ascontiguousarray(wt[:, sl].reshape(KC, P, OS, N)),
            "x": xp,
            "oh": oh,
        })
    return in_maps


def kernel(x, mx_train, scale, sigma, alpha, w, mx_start, _trace=False):
    global LAST_RESULTS
    from concourse.bass_utils import run_bass_kernel_spmd

    if "nc" not in _CACHE:
        _CACHE["nc"] = _build_nc()
    nc = _CACHE["nc"]
    in_maps = _prep_inputs(
        np.asarray(x, np.float32), np.asarray(mx_train, np.float32),
        np.asarray(scale, np.float32), np.asarray(sigma, np.float32),
        np.asarray(alpha, np.float32), np.asarray(w, np.float32),
        np.asarray(mx_start, np.float32),
    )
    res = run_bass_kernel_spmd(nc, in_maps, core_ids=list(range(NCORES)),
                               trace=_trace)
    LAST_RESULTS = res
    return np.concatenate([r["out"] for r in res.results], axis=1)
